# revision 1
# baseline (speedup 1.0000x reference)
"""ChebConv (K=4) GNN layer on 8 Trainium2 NeuronCores.

Strategy (sharding_hint: row-partition of the Laplacian + replicated weight):
  - Nodes V row-sharded across 8 cores (VS rows each, padded to VSP).
  - Each core owns the edges whose destination row lives in its shard.
  - The current poly y is replicated per core in a pair-transposed bf16
    layout xP[p, v, 2] = (y[v, 2p], y[v, 2p+1]); bitcast as f32 this makes
    every node a single f32 element per partition, so one GPSIMD ap_gather
    per edge-slab fetches all 256 features of y[col] for 128 partitions.
  - Each 128-edge group is PE-transposed back ([128 fp, 128 e] -> psum
    [128 e, 128 fp], the f32 pair moves as a unit so a bf16 bitcast yields
    z[e, 0:256] in feature order), then segment-summed into its 128-row
    tile with a one-hot matmul: S[e, r] = lap_val[e] * (row[e] == r),
    generated on-chip from an iota tile + tensor_scalar(is_equal, mult).
  - Chebyshev recurrence per row-tile on the vector engine; the new shard
    is written back pair-transposed (two strided PE transposes) and
    AllGathered to rebuild the replica; a pure-transposed fp32/bf16 copy
    feeds the final einsum (T0 comes pre-transposed from the host).
  - Final einsum contracts T_k with the weight on the PE, bias added
    per-partition, output written transposed and fixed up on host.

The instruction stream is identical on all cores (SPMD): per-(row-tile,
node-chunk) edge-cell sizes are padded to the max across cores, so only
the index/one-hot *data* differs per core.
"""

import sys

import numpy as np

sys.path.insert(0, "/opt/trn_rl_repo")

import ml_dtypes  # noqa: E402

BF16 = ml_dtypes.bfloat16


def make_cfg(V=100000, E=1600000, B=4, FIN=64, FOUT=64, NC=8, RT=128, BLK=4,
             NCC=4, GMAX=2048):
    VS = V // NC
    assert VS * NC == V
    VSP = ((VS + RT - 1) // RT) * RT
    NT = VSP // RT
    VG = VSP * NC
    assert NC % NCC == 0
    CHUNK = VG // NCC          # nodes per gather chunk (slab)
    CPB = NC // NCC            # core-blocks per chunk
    assert CHUNK == CPB * VSP
    assert CHUNK <= 32768      # ap_gather num_elems limit (f32, d=1)
    D = B * FIN
    return dict(V=V, E=E, B=B, FIN=FIN, FOUT=FOUT, NC=NC, RT=RT, BLK=BLK,
                CHUNK=CHUNK, GMAX=GMAX, VS=VS, VSP=VSP, NT=NT, VG=VG,
                NCC=NCC, CPB=CPB, D=D)


def _wrap16(idx, npart=128):
    """Pack an idx list (len n, multiple of 16) into the ap_gather layout:
    idx i at partition i%16, slot i//16, replicated to all 16-partition
    groups."""
    n = idx.shape[0]
    w = idx.reshape(n // 16, 16).T  # [16, n/16]
    return np.tile(w, (npart // 16, 1))


def preprocess(rows, cols, vals, cfg):
    """Build the static SPMD schedule + per-core index/one-hot data.

    Returns (prog, per_core): prog is core-independent structure;
    per_core[c] has 'gidx' [128, NIDX/16] int16 and 'meta' [128, NG, 2] f32.
    """
    NC, VS, VSP, RT, NT, BLK = cfg["NC"], cfg["VS"], cfg["VSP"], cfg["RT"], cfg["NT"], cfg["BLK"]
    CHUNK, GMAX, NCC = cfg["CHUNK"], cfg["GMAX"], cfg["NCC"]

    rows = np.asarray(rows, dtype=np.int64)
    cols = np.asarray(cols, dtype=np.int64)
    vals = np.asarray(vals, dtype=np.float32)

    owner = rows // VS
    lr = rows - owner * VS
    rt = lr // RT
    rloc = lr - rt * RT
    gc = (cols // VS) * VSP + (cols % VS)   # padded-global gather index
    cc = gc // CHUNK
    ci = (gc - cc * CHUNK).astype(np.int64)  # local node idx within chunk

    # per-core edge cells keyed by (rt, cc)
    cell_of = rt * NCC + cc
    ncells = NT * NCC
    counts = np.zeros((NC, ncells), dtype=np.int64)
    for c in range(NC):
        m = owner == c
        counts[c] = np.bincount(cell_of[m], minlength=ncells)
    mx = counts.max(axis=0)
    mpad = ((mx + RT - 1) // RT) * RT  # padded cell size, common to all cores
    # every rt needs at least one group so its PSUM accumulator exists
    mpad2 = mpad.reshape(NT, NCC)
    for t in range(NT):
        if mpad2[t].sum() == 0:
            mpad2[t, 0] = RT

    per_core_cells = []
    for c in range(NC):
        m = owner == c
        order = np.argsort(cell_of[m], kind="stable")
        e_ci = ci[m][order]
        e_rloc = rloc[m][order]
        e_val = vals[m][order]
        e_cell = cell_of[m][order]
        starts = np.searchsorted(e_cell, np.arange(ncells))
        ends = np.searchsorted(e_cell, np.arange(ncells) + 1)
        per_core_cells.append((e_ci, e_rloc, e_val, starts, ends))

    # stream order: for cc: for rt (chunk-outer so the slab loads once)
    NIDX = int(mpad2.sum())
    NG = NIDX // RT

    gidx = [np.zeros(NIDX, dtype=np.int16) for _ in range(NC)]
    gridx = [np.zeros((NG, RT), dtype=np.float32) for _ in range(NC)]
    gval = [np.zeros((NG, RT), dtype=np.float32) for _ in range(NC)]

    prog_phases = []
    seen_rt = set()
    pos = 0
    gpos = 0
    for ch in range(NCC):
        seg_groups = []
        for t in range(NT):
            n = int(mpad2[t, ch])
            if n == 0:
                continue
            for c in range(NC):
                e_ci, e_rloc, e_val, starts, ends = per_core_cells[c]
                s_, e_ = starts[t * NCC + ch], ends[t * NCC + ch]
                k = e_ - s_
                gidx[c][pos:pos + k] = e_ci[s_:e_].astype(np.int16)
                gr = gridx[c][gpos:gpos + n // RT].reshape(-1)
                gv = gval[c][gpos:gpos + n // RT].reshape(-1)
                gr[:k] = e_rloc[s_:e_].astype(np.float32)
                gv[:k] = e_val[s_:e_].astype(np.float32)
            ngr = n // RT
            for j in range(ngr):
                seg_groups.append({"g": gpos + j, "rt": t,
                                   "start": j == 0, "stop": j == ngr - 1,
                                   "acc": None})
            # cell ends -> accumulate psum into acc
            seg_groups[-1]["acc"] = "copy" if t not in seen_rt else "add"
            seen_rt.add(t)
            pos += n
            gpos += ngr
        calls = []
        gi = 0
        off0 = pos - len(seg_groups) * RT
        while gi < len(seg_groups):
            take = min(GMAX // RT, len(seg_groups) - gi)
            calls.append({"idx_off": off0 + gi * RT, "n": take * RT,
                          "groups": seg_groups[gi:gi + take]})
            gi += take
        prog_phases.append({"cc": ch, "calls": calls,
                            "idx_off": off0, "idx_n": len(seg_groups) * RT})
    assert pos == NIDX and gpos == NG
    assert len(seen_rt) == NT

    per_core = []
    for c in range(NC):
        meta = np.zeros((128, NG, 2), dtype=np.float32)
        meta[:, :, 0] = gridx[c].T
        meta[:, :, 1] = gval[c].T
        per_core.append({"gidx": _wrap16(gidx[c]), "meta": meta})

    prog = {"NIDX": NIDX, "NG": NG, "phases": prog_phases}
    return prog, per_core


def build_nc(cfg, prog, ag_mode="collective", skip=(), bufs=None):
    bufs = {**dict(zt=2, zg=3, st=3, pt=2), **(bufs or {})}
    import concourse.bacc as bacc
    import concourse.mybir as mybir
    import concourse.tile as tile

    NC, VSP, VG, D, RT, NT = cfg["NC"], cfg["VSP"], cfg["VG"], cfg["D"], cfg["RT"], cfg["NT"]
    CHUNK, NCC, CPB, B, FOUT = cfg["CHUNK"], cfg["NCC"], cfg["CPB"], cfg["B"], cfg["FOUT"]
    NIDX, NG = prog["NIDX"], prog["NG"]
    f32, bf16, i16 = mybir.dt.float32, mybir.dt.bfloat16, mybir.dt.int16
    HP = D // 2  # feature pairs = partitions of the paired replica
    AG_GROUPS = [list(range(NC))]

    nc = bacc.Bacc("TRN2", target_bir_lowering=False, debug=False, num_devices=NC)

    # inputs
    xg0P = nc.dram_tensor("xg0P", [NC * HP, VSP, 2], bf16, kind="ExternalInput")
    x0s = nc.dram_tensor("x0s", [VSP, D], f32, kind="ExternalInput")
    x0t = nc.dram_tensor("x0t", [D, VSP], f32, kind="ExternalInput")
    gidx_d = nc.dram_tensor("gidx", [128, NIDX // 16], i16, kind="ExternalInput")
    meta_d = nc.dram_tensor("meta", [128, NG, 2], f32, kind="ExternalInput")
    iota_d = nc.dram_tensor("iota", [128, RT], bf16, kind="ExternalInput")
    ident_d = nc.dram_tensor("ident", [128, 128], f32, kind="ExternalInput")
    w0_d = nc.dram_tensor("w0", [cfg["FIN"], FOUT], f32, kind="ExternalInput")
    wb_d = nc.dram_tensor("wb", [cfg["FIN"], 3, FOUT], bf16, kind="ExternalInput")
    bias_d = nc.dram_tensor("biasin", [FOUT, 1], f32, kind="ExternalInput")

    # outputs
    outT = nc.dram_tensor("outT", [B, FOUT, VSP], f32, kind="ExternalOutput")

    # internal DRAM
    xbP = [nc.dram_tensor(f"xbP{k}", [HP, VSP, 2], bf16) for k in (1, 2)]
    xfP = [nc.dram_tensor(f"xfP{k}", [NC * HP, VSP, 2], bf16,
                          addr_space="Shared") for k in (1, 2)]
    xt = [nc.dram_tensor(f"xt{k}", [D, VSP], bf16) for k in (1, 2, 3)]
    xs1 = nc.dram_tensor("xs1", [VSP, D], f32)

    with tile.TileContext(nc) as tc:
        with (
            tc.tile_pool(name="static", bufs=1) as sp,
            tc.tile_pool(name="accp", bufs=1) as acp,
            tc.tile_pool(name="slabp", bufs=1) as slp,
            tc.tile_pool(name="idxp", bufs=2) as ixp,
            tc.tile_pool(name="work", bufs=bufs["zt"]) as wp,
            tc.tile_pool(name="zgp", bufs=bufs["zg"]) as zgp,
            tc.tile_pool(name="stile", bufs=bufs["st"]) as stp,
            tc.tile_pool(name="fin", bufs=2) as fp,
            tc.tile_pool(name="psum", bufs=1, space="PSUM") as pp,
            tc.tile_pool(name="pst", bufs=bufs["pt"], space="PSUM") as ppt,
        ):
            meta_t = sp.tile([128, NG, 2], f32)
            iota_t = sp.tile([128, RT], bf16)
            ident_t = sp.tile([128, 128], f32)
            nc.sync.dma_start(out=meta_t[:], in_=meta_d[:])
            nc.sync.dma_start(out=iota_t[:], in_=iota_d[:])
            nc.sync.dma_start(out=ident_t[:], in_=ident_d[:])

            def spmm_round(r):
                src = xg0P if r == 1 else xfP[r - 2]
                acc = acp.tile([128, NT, D], bf16, tag="acc",
                               name=f"acc_r{r}")
                for ph in prog["phases"]:
                    ch = ph["cc"]
                    slab = slp.tile([128, CHUNK, 2], bf16, tag="slab",
                                    name=f"slab_r{r}_c{ch}")
                    for j in range(CPB):
                        c0 = (ch * CPB + j) * HP
                        nc.sync.dma_start(
                            out=slab[:, j * VSP:(j + 1) * VSP, :],
                            in_=src[c0:c0 + HP, :, :])
                    slab_f32 = slab[:].bitcast(f32)  # [128, CHUNK, 1]
                    io0, ion = ph["idx_off"], ph["idx_n"]
                    idx_t = ixp.tile([128, max(ion // 16, 16)], i16, tag="idx",
                                     name=f"idx_r{r}_c{ch}")
                    nc.sync.dma_start(
                        out=idx_t[:, :ion // 16],
                        in_=gidx_d[:, io0 // 16:(io0 + ion) // 16])
                    ps = None
                    for call in ph["calls"]:
                        n = call["n"]
                        zt = wp.tile([128, cfg["GMAX"]], f32, tag="zt")
                        o0 = call["idx_off"] - io0
                        idx_ap = idx_t[:, o0 // 16:(o0 + n) // 16]
                        if "gather" not in skip:
                            nc.gpsimd.ap_gather(
                                zt[:, :n], slab_f32, idx_ap,
                                channels=128, num_elems=CHUNK, d=1, num_idxs=n)
                        else:
                            nc.gpsimd.memset(zt[:, :n], 0.0)
                        for j, grp in enumerate(call["groups"]):
                            g, t = grp["g"], grp["rt"]
                            if grp["start"]:
                                ps = pp.tile([128, D], f32,
                                             tag=f"ps{t % cfg['BLK']}",
                                             name=f"ps_r{r}_t{t}_c{ch}")
                            zg = zgp.tile([128, 128], f32, tag="zg",
                                          name=f"zg_{r}_{g}")
                            if "tr" not in skip:
                                pt = ppt.tile([128, 128], f32, tag="pt",
                                              name=f"pt_{r}_{g}")
                                nc.tensor.transpose(
                                    pt[:], zt[:, j * RT:(j + 1) * RT], ident_t[:])
                                nc.any.tensor_copy(zg[:], pt[:])
                            else:
                                nc.vector.memset(zg[:], 0.0)
                            st = stp.tile([128, RT], bf16, tag="st",
                                          name=f"st_{r}_{g}")
                            nc.any.tensor_scalar(
                                st[:], iota_t[:],
                                meta_t[:, g, 0:1], meta_t[:, g, 1:2],
                                op0=mybir.AluOpType.is_equal,
                                op1=mybir.AluOpType.mult)
                            nc.tensor.matmul(
                                ps[:], st[:], zg[:].bitcast(bf16),
                                start=grp["start"], stop=grp["stop"])
                            if grp["acc"] == "copy":
                                nc.any.tensor_copy(acc[:, t, :], ps[:])
                            elif grp["acc"] == "add":
                                nc.any.tensor_tensor(
                                    acc[:, t, :], ps[:], acc[:, t, :],
                                    op=mybir.AluOpType.add)
                # finalize all row tiles
                for t in range(NT):
                    xnew = fp.tile([128, D], f32, tag="xnew",
                                   name=f"xnew_{r}_{t}")
                    if r == 1:
                        nc.vector.tensor_copy(xnew[:], acc[:, t, :])
                    else:
                        xprev_src = x0s if r == 2 else xs1
                        xp = fp.tile([128, D], f32, tag="xp",
                                     name=f"xp_{r}_{t}")
                        nc.sync.dma_start(
                            out=xp[:], in_=xprev_src[t * RT:(t + 1) * RT, :])
                        nc.vector.scalar_tensor_tensor(
                            xnew[:], acc[:, t, :], 2.0, xp[:],
                            op0=mybir.AluOpType.mult,
                            op1=mybir.AluOpType.subtract)
                    if r == 1:
                        nc.sync.dma_start(
                            out=xs1[t * RT:(t + 1) * RT, :], in_=xnew[:])
                    # pair-transpose: even/odd feature planes
                    pa = ppt.tile([128, 128], f32, tag="pt",
                                  name=f"pa_{r}_{t}")
                    pb = ppt.tile([128, 128], f32, tag="pt",
                                  name=f"pb_{r}_{t}")
                    xe = xnew[:].rearrange("v (f two) -> v two f", two=2)
                    nc.tensor.transpose(pa[:], xe[:, 0, :], ident_t[:])
                    nc.tensor.transpose(pb[:], xe[:, 1, :], ident_t[:])
                    # paired replica shard (gather source layout)
                    if r <= 2:
                        xtt = fp.tile([128, 128, 2], bf16, tag="xtt",
                                      name=f"xtt_{r}_{t}")
                        nc.any.tensor_copy(xtt[:, :, 0], pa[:])
                        nc.any.tensor_copy(xtt[:, :, 1], pb[:])
                        nc.sync.dma_start(
                            out=xbP[r - 1][:, t * RT:(t + 1) * RT, :],
                            in_=xtt[:])
                    # pure transposed copy for the einsum: feature row
                    # 2p+j is partition p of plane j
                    xtp = fp.tile([128, 2, 128], bf16, tag="xtp",
                                  name=f"xtp_{r}_{t}")
                    nc.any.tensor_copy(xtp[:, 0, :], pa[:])
                    nc.any.tensor_copy(xtp[:, 1, :], pb[:])
                    nc.sync.dma_start(
                        out=xt[r - 1].rearrange(
                            "(f two) v -> f two v", two=2)[:, :, t * RT:(t + 1) * RT],
                        in_=xtp[:])
                if r <= 2:
                    if ag_mode == "collective":
                        nc.gpsimd.collective_compute(
                            "AllGather", mybir.AluOpType.bypass,
                            replica_groups=AG_GROUPS,
                            ins=[xbP[r - 1][:]], outs=[xfP[r - 1][:]])
                    else:  # single-core timing stand-in: same DRAM traffic
                        for c in range(NC):
                            nc.sync.dma_start(
                                out=xfP[r - 1][c * HP:(c + 1) * HP, :, :],
                                in_=xbP[r - 1][:])

            for r in (1, 2, 3):
                spmm_round(r)

        # einsum: outT[b][o, v] = sum_k W_k^T @ T_k^T[b-rows, v] + bias
        with (
            tc.tile_pool(name="ew", bufs=1) as ewp,
            tc.tile_pool(name="erhs", bufs=3) as erp,
            tc.tile_pool(name="eout", bufs=3) as eop,
            tc.tile_pool(name="epsum", bufs=1, space="PSUM") as epp,
        ):
            w0_t = ewp.tile([cfg["FIN"], FOUT], f32)
            wb_t = ewp.tile([cfg["FIN"], 3, FOUT], bf16)
            bias_t = ewp.tile([FOUT, 1], f32)
            nc.sync.dma_start(out=w0_t[:], in_=w0_d[:])
            nc.sync.dma_start(out=wb_t[:], in_=wb_d[:])
            nc.sync.dma_start(out=bias_t[:], in_=bias_d[:])
            VC = 512
            nvc = (VSP + VC - 1) // VC
            for v in range(nvc):
                v0 = v * VC
                vn = min(VC, VSP - v0)
                for bb in range(B):
                    f0 = bb * cfg["FIN"]
                    r0 = erp.tile([cfg["FIN"], VC], f32, tag="r0",
                                  name=f"r0_{v}_{bb}")
                    nc.sync.dma_start(
                        out=r0[:, :vn], in_=x0t[f0:f0 + cfg["FIN"], v0:v0 + vn])
                    rk = {}
                    for k in (1, 2, 3):
                        rt_ = erp.tile([cfg["FIN"], VC], bf16, tag=f"rk{k}",
                                       name=f"rk_{v}_{bb}_{k}")
                        nc.sync.dma_start(
                            out=rt_[:, :vn],
                            in_=xt[k - 1][f0:f0 + cfg["FIN"], v0:v0 + vn])
                        rk[k] = rt_
                    ops = epp.tile([FOUT, VC], f32, tag=f"eps{bb % 4}",
                                   name=f"eps_{v}_{bb}")
                    nc.tensor.matmul(ops[:, :vn], w0_t[:], r0[:, :vn],
                                     start=True, stop=False)
                    for k in (1, 2, 3):
                        nc.tensor.matmul(ops[:, :vn], wb_t[:, k - 1, :],
                                         rk[k][:, :vn],
                                         start=False, stop=(k == 3))
                    ot = eop.tile([FOUT, VC], f32, tag="ot",
                                  name=f"ot_{v}_{bb}")
                    nc.vector.tensor_scalar(
                        ot[:, :vn], ops[:, :vn], bias_t[:], None,
                        op0=mybir.AluOpType.add)
                    nc.sync.dma_start(out=outT[bb][:, v0:v0 + vn], in_=ot[:, :vn])

    nc.compile()
    return nc


def _host_prep(x, weight, bias, lap_vals, lap_rows, lap_cols, cfg):
    NC, VS, VSP, VG, D = cfg["NC"], cfg["VS"], cfg["VSP"], cfg["VG"], cfg["D"]
    V = cfg["V"]
    HP = D // 2
    x = np.asarray(x, dtype=np.float32)
    x0 = np.ascontiguousarray(x.transpose(1, 0, 2).reshape(V, D))  # [V, B*FIN]

    prog, per_core = preprocess(lap_rows, lap_cols, lap_vals, cfg)

    x0p = np.zeros((VG, D), dtype=np.float32)
    for c in range(NC):
        x0p[c * VSP:c * VSP + VS] = x0[c * VS:(c + 1) * VS]
    # pair-transposed stacked replica: block c rows = HP feature-pairs
    xg0P = np.ascontiguousarray(
        x0p.reshape(NC, VSP, HP, 2).transpose(0, 2, 1, 3)
    ).reshape(NC * HP, VSP, 2).astype(BF16)

    iota = np.tile(np.arange(cfg["RT"], dtype=np.float32).astype(BF16), (128, 1))
    ident = np.eye(128, dtype=np.float32)
    weight = np.asarray(weight, dtype=np.float32)
    w0 = weight[0]
    wb = np.zeros((cfg["FIN"], 3, cfg["FOUT"]), dtype=BF16)
    for k in (1, 2, 3):
        wb[:, k - 1] = weight[k].astype(BF16)
    bias_in = np.asarray(bias, dtype=np.float32).reshape(cfg["FOUT"], 1)

    in_maps = []
    for c in range(NC):
        x0sh = x0p[c * VSP:(c + 1) * VSP]
        in_maps.append({
            "xg0P": xg0P,
            "x0s": np.ascontiguousarray(x0sh),
            "x0t": np.ascontiguousarray(x0sh.T),
            "gidx": per_core[c]["gidx"],
            "meta": per_core[c]["meta"],
            "iota": iota,
            "ident": ident,
            "w0": w0,
            "wb": wb,
            "biasin": bias_in,
        })
    return prog, in_maps


def _assemble(results, cfg):
    NC, VS, VSP, B, FOUT, V = (cfg["NC"], cfg["VS"], cfg["VSP"], cfg["B"],
                               cfg["FOUT"], cfg["V"])
    out = np.empty((B, V, FOUT), dtype=np.float32)
    for c in range(NC):
        oT = np.asarray(results[c]["outT"]).reshape(B, FOUT, VSP)
        out[:, c * VS:(c + 1) * VS, :] = oT.transpose(0, 2, 1)[:, :VS, :]
    return out


def run(x, weight, bias, lap_vals, lap_rows, lap_cols, trace=False):
    """Returns (output, BassKernelResults)."""
    from concourse import bass_utils

    cfg = make_cfg()
    prog, in_maps = _host_prep(x, weight, bias, lap_vals, lap_rows, lap_cols, cfg)
    nc = build_nc(cfg, prog)
    res = bass_utils.run_bass_kernel_spmd(nc, in_maps, list(range(cfg["NC"])),
                                          trace=trace)
    return _assemble(res.results, cfg), res


def kernel(x, weight, bias, lap_vals, lap_rows, lap_cols):
    out, _ = run(x, weight, bias, lap_vals, lap_rows, lap_cols)
    return out



# revision 2
# speedup vs baseline: 2.3407x; 2.3407x over previous
"""ChebConv (K=4) GNN layer on 8 Trainium2 NeuronCores.

Strategy (sharding_hint: row-partition of the Laplacian + replicated weight):
  - Nodes V row-sharded across 8 cores (VS rows each, padded to VSP).
  - Each core owns the edges whose destination row lives in its shard.
  - The current poly y is replicated per core in a pair-transposed bf16
    layout xP[p, v, 2] = (y[v, 2p], y[v, 2p+1]); bitcast as f32 this makes
    every node a single f32 element per partition, so one GPSIMD ap_gather
    per edge-slab fetches all 256 features of y[col] for 128 partitions.
  - Each 128-edge group is PE-transposed back ([128 fp, 128 e] -> psum
    [128 e, 128 fp], the f32 pair moves as a unit so a bf16 bitcast yields
    z[e, 0:256] in feature order), then segment-summed into its 128-row
    tile with a one-hot matmul: S[e, r] = lap_val[e] * (row[e] == r),
    generated on-chip from an iota tile + tensor_scalar(is_equal, mult).
  - Chebyshev recurrence per row-tile on the vector engine; the new shard
    is written back pair-transposed (two strided PE transposes) and
    AllGathered to rebuild the replica; a pure-transposed fp32/bf16 copy
    feeds the final einsum (T0 comes pre-transposed from the host).
  - Final einsum contracts T_k with the weight on the PE, bias added
    per-partition, output written transposed and fixed up on host.

The instruction stream is identical on all cores (SPMD): per-(row-tile,
node-chunk) edge-cell sizes are padded to the max across cores, so only
the index/one-hot *data* differs per core.
"""

import sys

import numpy as np

sys.path.insert(0, "/opt/trn_rl_repo")

import ml_dtypes  # noqa: E402

BF16 = ml_dtypes.bfloat16


def make_cfg(V=100000, E=1600000, B=4, FIN=64, FOUT=64, NC=8, RT=128, BLK=4,
             NCC=4, GMAX=2048):
    VS = V // NC
    assert VS * NC == V
    VSP = ((VS + RT - 1) // RT) * RT
    NT = VSP // RT
    VG = VSP * NC
    assert NC % NCC == 0
    CHUNK = VG // NCC          # nodes per gather chunk (slab)
    CPB = NC // NCC            # core-blocks per chunk
    assert CHUNK == CPB * VSP
    assert CHUNK <= 32768      # ap_gather num_elems limit (f32, d=1)
    D = B * FIN
    return dict(V=V, E=E, B=B, FIN=FIN, FOUT=FOUT, NC=NC, RT=RT, BLK=BLK,
                CHUNK=CHUNK, GMAX=GMAX, VS=VS, VSP=VSP, NT=NT, VG=VG,
                NCC=NCC, CPB=CPB, D=D)


def _wrap16(idx, npart=128):
    """Pack an idx list (len n, multiple of 16) into the ap_gather layout:
    idx i at partition i%16, slot i//16, replicated to all 16-partition
    groups."""
    n = idx.shape[0]
    w = idx.reshape(n // 16, 16).T  # [16, n/16]
    return np.tile(w, (npart // 16, 1))


def preprocess(rows, cols, vals, cfg):
    """Build the static SPMD schedule + per-core index/one-hot data.

    Returns (prog, per_core): prog is core-independent structure;
    per_core[c] has 'gidx' [128, NIDX/16] int16 and 'meta' [128, NG, 2] f32.
    """
    NC, VS, VSP, RT, NT, BLK = cfg["NC"], cfg["VS"], cfg["VSP"], cfg["RT"], cfg["NT"], cfg["BLK"]
    CHUNK, GMAX, NCC = cfg["CHUNK"], cfg["GMAX"], cfg["NCC"]

    rows = np.asarray(rows, dtype=np.int64)
    cols = np.asarray(cols, dtype=np.int64)
    vals = np.asarray(vals, dtype=np.float32)

    owner = rows // VS
    lr = rows - owner * VS
    rt = lr // RT
    rloc = lr - rt * RT
    gc = (cols // VS) * VSP + (cols % VS)   # padded-global gather index
    cc = gc // CHUNK
    ci = (gc - cc * CHUNK).astype(np.int64)  # local node idx within chunk

    # per-core edge cells keyed by (rt, cc)
    cell_of = rt * NCC + cc
    ncells = NT * NCC
    counts = np.zeros((NC, ncells), dtype=np.int64)
    for c in range(NC):
        m = owner == c
        counts[c] = np.bincount(cell_of[m], minlength=ncells)
    mx = counts.max(axis=0)
    mpad = ((mx + RT - 1) // RT) * RT  # padded cell size, common to all cores
    # every rt needs at least one group so its PSUM accumulator exists
    mpad2 = mpad.reshape(NT, NCC)
    for t in range(NT):
        if mpad2[t].sum() == 0:
            mpad2[t, 0] = RT

    per_core_cells = []
    for c in range(NC):
        m = owner == c
        order = np.argsort(cell_of[m], kind="stable")
        e_ci = ci[m][order]
        e_rloc = rloc[m][order]
        e_val = vals[m][order]
        e_cell = cell_of[m][order]
        starts = np.searchsorted(e_cell, np.arange(ncells))
        ends = np.searchsorted(e_cell, np.arange(ncells) + 1)
        per_core_cells.append((e_ci, e_rloc, e_val, starts, ends))

    # stream order: for cc: for rt (chunk-outer so the slab loads once)
    NIDX = int(mpad2.sum())
    NG = NIDX // RT

    gidx = [np.zeros(NIDX, dtype=np.int16) for _ in range(NC)]
    gridx = [np.zeros((NG, RT), dtype=np.float32) for _ in range(NC)]
    gval = [np.zeros((NG, RT), dtype=np.float32) for _ in range(NC)]

    prog_phases = []
    seen_rt = set()
    pos = 0
    gpos = 0
    for ch in range(NCC):
        seg_groups = []
        for t in range(NT):
            n = int(mpad2[t, ch])
            if n == 0:
                continue
            for c in range(NC):
                e_ci, e_rloc, e_val, starts, ends = per_core_cells[c]
                s_, e_ = starts[t * NCC + ch], ends[t * NCC + ch]
                k = e_ - s_
                gidx[c][pos:pos + k] = e_ci[s_:e_].astype(np.int16)
                gr = gridx[c][gpos:gpos + n // RT].reshape(-1)
                gv = gval[c][gpos:gpos + n // RT].reshape(-1)
                gr[:k] = e_rloc[s_:e_].astype(np.float32)
                gv[:k] = e_val[s_:e_].astype(np.float32)
            ngr = n // RT
            for j in range(ngr):
                seg_groups.append({"g": gpos + j, "rt": t,
                                   "start": j == 0, "stop": j == ngr - 1,
                                   "acc": None})
            # cell ends -> accumulate psum into acc
            seg_groups[-1]["acc"] = "copy" if t not in seen_rt else "add"
            seen_rt.add(t)
            pos += n
            gpos += ngr
        calls = []
        gi = 0
        off0 = pos - len(seg_groups) * RT
        while gi < len(seg_groups):
            take = min(GMAX // RT, len(seg_groups) - gi)
            calls.append({"idx_off": off0 + gi * RT, "n": take * RT,
                          "groups": seg_groups[gi:gi + take]})
            gi += take
        prog_phases.append({"cc": ch, "calls": calls,
                            "idx_off": off0, "idx_n": len(seg_groups) * RT})
    assert pos == NIDX and gpos == NG
    assert len(seen_rt) == NT

    per_core = []
    for c in range(NC):
        meta = np.zeros((128, NG, 2), dtype=np.float32)
        meta[:, :, 0] = gridx[c].T
        meta[:, :, 1] = gval[c].T
        per_core.append({"gidx": _wrap16(gidx[c]), "meta": meta})

    prog = {"NIDX": NIDX, "NG": NG, "phases": prog_phases}
    return prog, per_core


def build_nc(cfg, prog, ag_mode="collective", skip=(), bufs=None):
    bufs = {**dict(zt=2, zg=3, st=3, pt=2), **(bufs or {})}
    import concourse.bacc as bacc
    import concourse.mybir as mybir
    import concourse.tile as tile

    NC, VSP, VG, D, RT, NT = cfg["NC"], cfg["VSP"], cfg["VG"], cfg["D"], cfg["RT"], cfg["NT"]
    CHUNK, NCC, CPB, B, FOUT = cfg["CHUNK"], cfg["NCC"], cfg["CPB"], cfg["B"], cfg["FOUT"]
    NIDX, NG = prog["NIDX"], prog["NG"]
    f32, bf16, i16 = mybir.dt.float32, mybir.dt.bfloat16, mybir.dt.int16
    HP = D // 2  # feature pairs = partitions of the paired replica
    AG_GROUPS = [list(range(NC))]

    nc = bacc.Bacc("TRN2", target_bir_lowering=False, debug=False, num_devices=NC)

    # inputs
    xg0P = nc.dram_tensor("xg0P", [NC * HP, VSP, 2], bf16, kind="ExternalInput")
    x0s = nc.dram_tensor("x0s", [VSP, D], f32, kind="ExternalInput")
    x0t = nc.dram_tensor("x0t", [D, VSP], f32, kind="ExternalInput")
    gidx_d = nc.dram_tensor("gidx", [128, NIDX // 16], i16, kind="ExternalInput")
    meta_d = nc.dram_tensor("meta", [128, NG, 2], f32, kind="ExternalInput")
    iota_d = nc.dram_tensor("iota", [128, RT], bf16, kind="ExternalInput")
    ident_d = nc.dram_tensor("ident", [128, 128], f32, kind="ExternalInput")
    w0_d = nc.dram_tensor("w0", [cfg["FIN"], FOUT], f32, kind="ExternalInput")
    wb_d = nc.dram_tensor("wb", [cfg["FIN"], 3, FOUT], bf16, kind="ExternalInput")
    bias_d = nc.dram_tensor("biasin", [FOUT, 1], f32, kind="ExternalInput")

    # outputs
    outT = nc.dram_tensor("outT", [B, FOUT, VSP], f32, kind="ExternalOutput")

    # internal DRAM
    xbP = [nc.dram_tensor(f"xbP{k}", [HP, VSP, 2], bf16) for k in (1, 2)]
    xfP = [nc.dram_tensor(f"xfP{k}", [NC * HP, VSP, 2], bf16,
                          addr_space="Shared") for k in (1, 2)]
    xt = [nc.dram_tensor(f"xt{k}", [D, VSP], bf16) for k in (1, 2, 3)]
    xs1 = nc.dram_tensor("xs1", [VSP, D], f32)

    with tile.TileContext(nc) as tc:
        with (
            tc.tile_pool(name="static", bufs=1) as sp,
            tc.tile_pool(name="accp", bufs=1) as acp,
            tc.tile_pool(name="slabp", bufs=1) as slp,
            tc.tile_pool(name="idxp", bufs=2) as ixp,
            tc.tile_pool(name="work", bufs=bufs["zt"]) as wp,
            tc.tile_pool(name="zgp", bufs=bufs["zg"]) as zgp,
            tc.tile_pool(name="stile", bufs=bufs["st"]) as stp,
            tc.tile_pool(name="fin", bufs=2) as fp,
            tc.tile_pool(name="psum", bufs=1, space="PSUM") as pp,
            tc.tile_pool(name="pst", bufs=bufs["pt"], space="PSUM") as ppt,
        ):
            meta_t = sp.tile([128, NG, 2], f32)
            iota_t = sp.tile([128, RT], bf16)
            ident_t = sp.tile([128, 128], f32)
            nc.sync.dma_start(out=meta_t[:], in_=meta_d[:])
            nc.sync.dma_start(out=iota_t[:], in_=iota_d[:])
            nc.sync.dma_start(out=ident_t[:], in_=ident_d[:])

            def spmm_round(r):
                src = xg0P if r == 1 else xfP[r - 2]
                acc = acp.tile([128, NT, D], bf16, tag="acc",
                               name=f"acc_r{r}")
                for ph in prog["phases"]:
                    ch = ph["cc"]
                    slab = slp.tile([128, CHUNK, 2], bf16, tag="slab",
                                    name=f"slab_r{r}_c{ch}")
                    for j in range(CPB):
                        c0 = (ch * CPB + j) * HP
                        nc.sync.dma_start(
                            out=slab[:, j * VSP:(j + 1) * VSP, :],
                            in_=src[c0:c0 + HP, :, :])
                    slab_f32 = slab[:].bitcast(f32)  # [128, CHUNK, 1]
                    io0, ion = ph["idx_off"], ph["idx_n"]
                    idx_t = ixp.tile([128, max(ion // 16, 16)], i16, tag="idx",
                                     name=f"idx_r{r}_c{ch}")
                    nc.sync.dma_start(
                        out=idx_t[:, :ion // 16],
                        in_=gidx_d[:, io0 // 16:(io0 + ion) // 16])
                    ps = None
                    for call in ph["calls"]:
                        n = call["n"]
                        zt = wp.tile([128, cfg["GMAX"]], f32, tag="zt")
                        o0 = call["idx_off"] - io0
                        idx_ap = idx_t[:, o0 // 16:(o0 + n) // 16]
                        if "gather" not in skip:
                            nc.gpsimd.ap_gather(
                                zt[:, :n], slab_f32, idx_ap,
                                channels=128, num_elems=CHUNK, d=1, num_idxs=n)
                        else:
                            nc.gpsimd.memset(zt[:, :n], 0.0)
                        groups = call["groups"]
                        # quads of groups share one PSUM bank + one copy
                        for q0 in range(0, len(groups), 4):
                            quad = groups[q0:q0 + 4]
                            nq = len(quad)
                            g0 = quad[0]["g"]
                            if "tr" not in skip:
                                ptq = ppt.tile([128, 512], f32, tag="pt",
                                               name=f"ptq_{r}_{g0}")
                                for qi in range(nq):
                                    j = q0 + qi
                                    nc.tensor.transpose(
                                        ptq[:, qi * RT:(qi + 1) * RT],
                                        zt[:, j * RT:(j + 1) * RT], ident_t[:])
                            zgq = zgp.tile([128, 512], f32, tag="zg",
                                           name=f"zgq_{r}_{g0}")
                            if "tr" not in skip:
                                nc.any.tensor_copy(zgq[:, :nq * RT],
                                                   ptq[:, :nq * RT])
                            else:
                                nc.vector.memset(zgq[:, :nq * RT], 0.0)
                            for qi, grp in enumerate(quad):
                                g, t = grp["g"], grp["rt"]
                                if grp["start"]:
                                    ps = pp.tile([128, D], f32,
                                                 tag=f"ps{t % cfg['BLK']}",
                                                 name=f"ps_r{r}_t{t}_c{ch}")
                                st = stp.tile([128, RT], bf16, tag="st",
                                              name=f"st_{r}_{g}")
                                nc.any.tensor_scalar(
                                    st[:], iota_t[:],
                                    meta_t[:, g, 0:1], meta_t[:, g, 1:2],
                                    op0=mybir.AluOpType.is_equal,
                                    op1=mybir.AluOpType.mult)
                                nc.tensor.matmul(
                                    ps[:], st[:],
                                    zgq[:, qi * RT:(qi + 1) * RT].bitcast(bf16),
                                    start=grp["start"], stop=grp["stop"])
                                if grp["acc"] == "copy":
                                    nc.any.tensor_copy(acc[:, t, :], ps[:])
                                elif grp["acc"] == "add":
                                    nc.any.tensor_tensor(
                                        acc[:, t, :], ps[:], acc[:, t, :],
                                        op=mybir.AluOpType.add)
                # finalize all row tiles
                for t in range(NT):
                    xnew = fp.tile([128, D], f32, tag="xnew",
                                   name=f"xnew_{r}_{t}")
                    if r == 1:
                        nc.vector.tensor_copy(xnew[:], acc[:, t, :])
                    else:
                        xprev_src = x0s if r == 2 else xs1
                        xp = fp.tile([128, D], f32, tag="xp",
                                     name=f"xp_{r}_{t}")
                        nc.sync.dma_start(
                            out=xp[:], in_=xprev_src[t * RT:(t + 1) * RT, :])
                        nc.vector.scalar_tensor_tensor(
                            xnew[:], acc[:, t, :], 2.0, xp[:],
                            op0=mybir.AluOpType.mult,
                            op1=mybir.AluOpType.subtract)
                    if r == 1:
                        nc.sync.dma_start(
                            out=xs1[t * RT:(t + 1) * RT, :], in_=xnew[:])
                    # pair-transpose: even/odd feature planes
                    pa = ppt.tile([128, 128], f32, tag="pt",
                                  name=f"pa_{r}_{t}")
                    pb = ppt.tile([128, 128], f32, tag="pt",
                                  name=f"pb_{r}_{t}")
                    xe = xnew[:].rearrange("v (f two) -> v two f", two=2)
                    nc.tensor.transpose(pa[:], xe[:, 0, :], ident_t[:])
                    nc.tensor.transpose(pb[:], xe[:, 1, :], ident_t[:])
                    # paired replica shard (gather source layout)
                    if r <= 2:
                        xtt = fp.tile([128, 128, 2], bf16, tag="xtt",
                                      name=f"xtt_{r}_{t}")
                        nc.any.tensor_copy(xtt[:, :, 0], pa[:])
                        nc.any.tensor_copy(xtt[:, :, 1], pb[:])
                        nc.sync.dma_start(
                            out=xbP[r - 1][:, t * RT:(t + 1) * RT, :],
                            in_=xtt[:])
                    # pure transposed copy for the einsum: feature row
                    # 2p+j is partition p of plane j
                    xtp = fp.tile([128, 2, 128], bf16, tag="xtp",
                                  name=f"xtp_{r}_{t}")
                    nc.any.tensor_copy(xtp[:, 0, :], pa[:])
                    nc.any.tensor_copy(xtp[:, 1, :], pb[:])
                    nc.sync.dma_start(
                        out=xt[r - 1].rearrange(
                            "(f two) v -> f two v", two=2)[:, :, t * RT:(t + 1) * RT],
                        in_=xtp[:])
                if r <= 2:
                    if ag_mode == "collective":
                        nc.gpsimd.collective_compute(
                            "AllGather", mybir.AluOpType.bypass,
                            replica_groups=AG_GROUPS,
                            ins=[xbP[r - 1][:]], outs=[xfP[r - 1][:]])
                    else:  # single-core timing stand-in: same DRAM traffic
                        for c in range(NC):
                            nc.sync.dma_start(
                                out=xfP[r - 1][c * HP:(c + 1) * HP, :, :],
                                in_=xbP[r - 1][:])

            for r in (1, 2, 3):
                spmm_round(r)

        # einsum: outT[b][o, v] = sum_k W_k^T @ T_k^T[b-rows, v] + bias
        with (
            tc.tile_pool(name="ew", bufs=1) as ewp,
            tc.tile_pool(name="erhs", bufs=3) as erp,
            tc.tile_pool(name="eout", bufs=3) as eop,
            tc.tile_pool(name="epsum", bufs=1, space="PSUM") as epp,
        ):
            w0_t = ewp.tile([cfg["FIN"], FOUT], f32)
            wb_t = ewp.tile([cfg["FIN"], 3, FOUT], bf16)
            bias_t = ewp.tile([FOUT, 1], f32)
            nc.sync.dma_start(out=w0_t[:], in_=w0_d[:])
            nc.sync.dma_start(out=wb_t[:], in_=wb_d[:])
            nc.sync.dma_start(out=bias_t[:], in_=bias_d[:])
            VC = 512
            nvc = (VSP + VC - 1) // VC
            for v in range(nvc):
                v0 = v * VC
                vn = min(VC, VSP - v0)
                for bb in range(B):
                    f0 = bb * cfg["FIN"]
                    r0 = erp.tile([cfg["FIN"], VC], f32, tag="r0",
                                  name=f"r0_{v}_{bb}")
                    nc.sync.dma_start(
                        out=r0[:, :vn], in_=x0t[f0:f0 + cfg["FIN"], v0:v0 + vn])
                    rk = {}
                    for k in (1, 2, 3):
                        rt_ = erp.tile([cfg["FIN"], VC], bf16, tag=f"rk{k}",
                                       name=f"rk_{v}_{bb}_{k}")
                        nc.sync.dma_start(
                            out=rt_[:, :vn],
                            in_=xt[k - 1][f0:f0 + cfg["FIN"], v0:v0 + vn])
                        rk[k] = rt_
                    ops = epp.tile([FOUT, VC], f32, tag=f"eps{bb % 4}",
                                   name=f"eps_{v}_{bb}")
                    nc.tensor.matmul(ops[:, :vn], w0_t[:], r0[:, :vn],
                                     start=True, stop=False)
                    for k in (1, 2, 3):
                        nc.tensor.matmul(ops[:, :vn], wb_t[:, k - 1, :],
                                         rk[k][:, :vn],
                                         start=False, stop=(k == 3))
                    ot = eop.tile([FOUT, VC], f32, tag="ot",
                                  name=f"ot_{v}_{bb}")
                    nc.vector.tensor_scalar(
                        ot[:, :vn], ops[:, :vn], bias_t[:], None,
                        op0=mybir.AluOpType.add)
                    nc.sync.dma_start(out=outT[bb][:, v0:v0 + vn], in_=ot[:, :vn])

    nc.compile()
    return nc


def _host_prep(x, weight, bias, lap_vals, lap_rows, lap_cols, cfg):
    NC, VS, VSP, VG, D = cfg["NC"], cfg["VS"], cfg["VSP"], cfg["VG"], cfg["D"]
    V = cfg["V"]
    HP = D // 2
    x = np.asarray(x, dtype=np.float32)
    x0 = np.ascontiguousarray(x.transpose(1, 0, 2).reshape(V, D))  # [V, B*FIN]

    prog, per_core = preprocess(lap_rows, lap_cols, lap_vals, cfg)

    x0p = np.zeros((VG, D), dtype=np.float32)
    for c in range(NC):
        x0p[c * VSP:c * VSP + VS] = x0[c * VS:(c + 1) * VS]
    # pair-transposed stacked replica: block c rows = HP feature-pairs
    xg0P = np.ascontiguousarray(
        x0p.reshape(NC, VSP, HP, 2).transpose(0, 2, 1, 3)
    ).reshape(NC * HP, VSP, 2).astype(BF16)

    iota = np.tile(np.arange(cfg["RT"], dtype=np.float32).astype(BF16), (128, 1))
    ident = np.eye(128, dtype=np.float32)
    weight = np.asarray(weight, dtype=np.float32)
    w0 = weight[0]
    wb = np.zeros((cfg["FIN"], 3, cfg["FOUT"]), dtype=BF16)
    for k in (1, 2, 3):
        wb[:, k - 1] = weight[k].astype(BF16)
    bias_in = np.asarray(bias, dtype=np.float32).reshape(cfg["FOUT"], 1)

    in_maps = []
    for c in range(NC):
        x0sh = x0p[c * VSP:(c + 1) * VSP]
        in_maps.append({
            "xg0P": xg0P,
            "x0s": np.ascontiguousarray(x0sh),
            "x0t": np.ascontiguousarray(x0sh.T),
            "gidx": per_core[c]["gidx"],
            "meta": per_core[c]["meta"],
            "iota": iota,
            "ident": ident,
            "w0": w0,
            "wb": wb,
            "biasin": bias_in,
        })
    return prog, in_maps


def _assemble(results, cfg):
    NC, VS, VSP, B, FOUT, V = (cfg["NC"], cfg["VS"], cfg["VSP"], cfg["B"],
                               cfg["FOUT"], cfg["V"])
    out = np.empty((B, V, FOUT), dtype=np.float32)
    for c in range(NC):
        oT = np.asarray(results[c]["outT"]).reshape(B, FOUT, VSP)
        out[:, c * VS:(c + 1) * VS, :] = oT.transpose(0, 2, 1)[:, :VS, :]
    return out


def run(x, weight, bias, lap_vals, lap_rows, lap_cols, trace=False):
    """Returns (output, BassKernelResults)."""
    from concourse import bass_utils

    cfg = make_cfg()
    prog, in_maps = _host_prep(x, weight, bias, lap_vals, lap_rows, lap_cols, cfg)
    nc = build_nc(cfg, prog)
    res = bass_utils.run_bass_kernel_spmd(nc, in_maps, list(range(cfg["NC"])),
                                          trace=trace)
    return _assemble(res.results, cfg), res


def kernel(x, weight, bias, lap_vals, lap_rows, lap_cols):
    out, _ = run(x, weight, bias, lap_vals, lap_rows, lap_cols)
    return out



# revision 5
# speedup vs baseline: 8.3847x; 3.5821x over previous
"""ChebConv (K=4) GNN layer on 8 Trainium2 NeuronCores.

Strategy (sharding_hint: row-partition of the Laplacian + replicated weight):
  - Nodes V row-sharded across 8 cores (VS rows each, padded to VSP=NT*128).
  - Each core owns the edges whose destination row lives in its shard.
  - The current poly y lives as a full node-major bf16 replica [VG, 256] in
    DRAM (AllGather of the 8 shards per spmm round). Edge gathers use
    GPSIMD-issued SWDGE dma_gather: each index pulls one node's 512B feature
    row from HBM straight into SBUF at partition i%128, slot i//128 — i.e.
    a [128 edges, 256 feat] matmul-ready block, no transposes.
  - Edges are ordered tile-major: for each block of TBLK row tiles, for each
    of the NCC=4 node chunks (int16 gather index range), the (tile, chunk)
    edge cells. Segment-sum is a one-hot matmul S[e, r]=val*(rloc==r)
    accumulated in a per-tile PSUM bank across the whole stream.
  - A row tile finalizes right after its last matmul: Chebyshev recurrence
    on the vector engine, bf16 shard write (AllGather input), and a pair of
    PE transposes producing the feature-major copy the final einsum needs.
    The AllGather is split in two halves so the first fires mid-round.
  - Final einsum contracts T_k with the replicated weight on the PE.

The instruction stream is identical on all cores (SPMD): per-(tile, chunk)
edge-cell sizes are padded to the max across cores, so only the index /
one-hot data differs per core.
"""

import sys

import numpy as np

sys.path.insert(0, "/opt/trn_rl_repo")

import ml_dtypes  # noqa: E402

BF16 = ml_dtypes.bfloat16


def make_cfg(V=100000, E=1600000, B=4, FIN=64, FOUT=64, NC=8, RT=128,
             NCC=4, TBLK=4, CALLMAX=1024, NQ=1):
    VS = V // NC
    assert VS * NC == V
    VSP = ((VS + RT - 1) // RT) * RT
    NT = VSP // RT
    VG = VSP * NC
    CHUNK = VG // NCC          # nodes per gather chunk (int16 idx range)
    assert CHUNK * NCC == VG
    assert CHUNK <= 32768      # int16 dma_gather row index limit
    D = B * FIN
    return dict(V=V, E=E, B=B, FIN=FIN, FOUT=FOUT, NC=NC, RT=RT,
                CHUNK=CHUNK, CALLMAX=CALLMAX, VS=VS, VSP=VSP, NT=NT, VG=VG,
                NCC=NCC, TBLK=TBLK, D=D, NQ=NQ)


def _wrap16(idx, npart=128):
    """Pack an idx list (len n, multiple of 16) into the SWDGE gather layout:
    idx i at partition i%16, slot i//16, replicated to all 16-partition
    groups."""
    n = idx.shape[0]
    w = idx.reshape(n // 16, 16).T  # [16, n/16]
    return np.tile(w, (npart // 16, 1))


def preprocess(rows, cols, vals, cfg):
    """Build the static SPMD schedule + per-core index/one-hot data.

    Returns (prog, per_core): prog is core-independent structure;
    per_core[c] has 'gidx' [128, NIDX/16] int16 and 'meta' [128, NG, 2] f32.
    """
    NC, VS, VSP, RT, NT = cfg["NC"], cfg["VS"], cfg["VSP"], cfg["RT"], cfg["NT"]
    CHUNK, CALLMAX, NCC, TBLK = (cfg["CHUNK"], cfg["CALLMAX"], cfg["NCC"],
                                 cfg["TBLK"])

    rows = np.asarray(rows, dtype=np.int64)
    cols = np.asarray(cols, dtype=np.int64)
    vals = np.asarray(vals, dtype=np.float32)

    owner = rows // VS
    lr = rows - owner * VS
    rt = lr // RT
    rloc = lr - rt * RT
    gc = (cols // VS) * VSP + (cols % VS)   # padded-global replica row
    cc = gc // CHUNK
    ci = (gc - cc * CHUNK).astype(np.int64)  # row idx within chunk

    # per-core edge cells keyed by (rt, cc)
    cell_of = rt * NCC + cc
    ncells = NT * NCC
    counts = np.zeros((NC, ncells), dtype=np.int64)
    for c in range(NC):
        m = owner == c
        counts[c] = np.bincount(cell_of[m], minlength=ncells)
    mx = counts.max(axis=0)
    mpad = ((mx + RT - 1) // RT) * RT  # padded cell size, common to all cores
    mpad2 = mpad.reshape(NT, NCC)
    # every tile needs at least one group so its PSUM accumulator exists
    for t in range(NT):
        if mpad2[t].sum() == 0:
            mpad2[t, 0] = RT

    per_core_cells = []
    for c in range(NC):
        m = owner == c
        order = np.argsort(cell_of[m], kind="stable")
        e_ci = ci[m][order]
        e_rloc = rloc[m][order]
        e_val = vals[m][order]
        e_cell = cell_of[m][order]
        starts = np.searchsorted(e_cell, np.arange(ncells))
        ends = np.searchsorted(e_cell, np.arange(ncells) + 1)
        per_core_cells.append((e_ci, e_rloc, e_val, starts, ends))

    NIDX = int(mpad2.sum())
    NG = NIDX // RT

    gidx = [np.zeros(NIDX, dtype=np.int16) for _ in range(NC)]
    gridx = [np.zeros((NG, RT), dtype=np.float32) for _ in range(NC)]
    gval = [np.zeros((NG, RT), dtype=np.float32) for _ in range(NC)]

    # per-tile first/last group for start/stop flags
    tile_ngroups = (mpad2 // RT).sum(axis=1)
    steps = []
    pos = 0
    gpos = 0
    for tb in range(0, NT, TBLK):
        tiles = list(range(tb, min(tb + TBLK, NT)))
        remaining = {t: int(tile_ngroups[t]) for t in tiles}
        started = set()
        for ch in range(NCC):
            # groups of cells (t, ch) for t in tiles, in tile order
            seg_groups = []
            for t in tiles:
                n = int(mpad2[t, ch])
                if n == 0:
                    continue
                for c in range(NC):
                    e_ci, e_rloc, e_val, starts_, ends_ = per_core_cells[c]
                    s_, e_ = starts_[t * NCC + ch], ends_[t * NCC + ch]
                    k = e_ - s_
                    gidx[c][pos:pos + k] = e_ci[s_:e_].astype(np.int16)
                    gr = gridx[c][gpos:gpos + n // RT].reshape(-1)
                    gv = gval[c][gpos:gpos + n // RT].reshape(-1)
                    gr[:k] = e_rloc[s_:e_].astype(np.float32)
                    gv[:k] = e_val[s_:e_].astype(np.float32)
                ngr = n // RT
                for j in range(ngr):
                    st = t not in started
                    started.add(t)
                    remaining[t] -= 1
                    seg_groups.append({"g": gpos + j, "rt": t, "start": st,
                                       "stop": remaining[t] == 0})
                pos += n
                gpos += ngr
            # pack groups into calls of <= CALLMAX idxs
            gi = 0
            off0 = pos - len(seg_groups) * RT
            while gi < len(seg_groups):
                take = min(CALLMAX // RT, len(seg_groups) - gi)
                grps = seg_groups[gi:gi + take]
                fin = [g["rt"] for g in grps if g["stop"]]
                steps.append({"cc": ch, "idx_off": off0 + gi * RT,
                              "n": take * RT, "groups": grps, "fin": fin})
                gi += take
    assert pos == NIDX and gpos == NG

    per_core = []
    for c in range(NC):
        meta = np.zeros((128, NG, 2), dtype=np.float32)
        meta[:, :, 0] = gridx[c].T
        meta[:, :, 1] = gval[c].T
        per_core.append({"gidx": _wrap16(gidx[c]), "meta": meta})

    prog = {"NIDX": NIDX, "NG": NG, "steps": steps}
    return prog, per_core


def build_nc(cfg, prog, ag_mode="collective", ag_split=1):
    import concourse.bacc as bacc
    import concourse.mybir as mybir
    import concourse.tile as tile

    NC, VSP, VG, D, RT, NT = (cfg["NC"], cfg["VSP"], cfg["VG"], cfg["D"],
                              cfg["RT"], cfg["NT"])
    CHUNK, B, FOUT, NQ = cfg["CHUNK"], cfg["B"], cfg["FOUT"], cfg["NQ"]
    NIDX, NG = prog["NIDX"], prog["NG"]
    f32, bf16, i16 = mybir.dt.float32, mybir.dt.bfloat16, mybir.dt.int16
    AG_GROUPS = [list(range(NC))]
    # AllGather split boundaries (by row tile)
    ag_bounds = [NT * (i + 1) // ag_split for i in range(ag_split)]

    nc = bacc.Bacc("TRN2", target_bir_lowering=False, debug=False,
                   num_devices=NC, num_swdge_queues=NQ)

    # inputs
    xg0 = nc.dram_tensor("xg0", [VG, D], bf16, kind="ExternalInput")
    x0s = nc.dram_tensor("x0s", [VSP, D], f32, kind="ExternalInput")
    x0t = nc.dram_tensor("x0t", [D, VSP], f32, kind="ExternalInput")
    gidx_d = nc.dram_tensor("gidx", [128, NIDX // 16], i16, kind="ExternalInput")
    meta_d = nc.dram_tensor("meta", [128, NG, 2], f32, kind="ExternalInput")
    iota_d = nc.dram_tensor("iota", [128, RT], bf16, kind="ExternalInput")
    ident_d = nc.dram_tensor("ident", [128, 128], f32, kind="ExternalInput")
    w0_d = nc.dram_tensor("w0", [cfg["FIN"], FOUT], f32, kind="ExternalInput")
    wb_d = nc.dram_tensor("wb", [cfg["FIN"], 3, FOUT], bf16, kind="ExternalInput")
    bias_d = nc.dram_tensor("biasin", [FOUT, 1], f32, kind="ExternalInput")

    # outputs
    outT = nc.dram_tensor("outT", [B, FOUT, VSP], f32, kind="ExternalOutput")

    # internal DRAM
    xb = [nc.dram_tensor(f"xb{k}", [VSP, D], bf16) for k in (1, 2)]
    xf = [nc.dram_tensor(f"xf{k}", [VG, D], bf16, addr_space="Shared")
          for k in (1, 2)]
    xt = [nc.dram_tensor(f"xt{k}", [D, VSP], bf16) for k in (1, 2, 3)]
    xs1 = nc.dram_tensor("xs1", [VSP, D], f32)

    qn = [0]

    with tile.TileContext(nc) as tc:
        with (
            tc.tile_pool(name="static", bufs=1) as sp,
            tc.tile_pool(name="zqp", bufs=4) as zqp,
            tc.tile_pool(name="stile", bufs=4) as stp,
            tc.tile_pool(name="fin", bufs=3) as fp,
            tc.tile_pool(name="psum", bufs=1, space="PSUM") as pp,
            tc.tile_pool(name="pab", bufs=2, space="PSUM") as pabp,
        ):
            gidx_t = sp.tile([128, NIDX // 16], i16)
            meta_t = sp.tile([128, NG, 2], f32)
            iota_t = sp.tile([128, RT], bf16)
            ident_t = sp.tile([128, 128], f32)
            nc.sync.dma_start(out=gidx_t[:], in_=gidx_d[:])
            nc.sync.dma_start(out=meta_t[:], in_=meta_d[:])
            nc.sync.dma_start(out=iota_t[:], in_=iota_d[:])
            nc.sync.dma_start(out=ident_t[:], in_=ident_d[:])

            def finalize(r, t, ps):
                xnew = fp.tile([128, D], f32, tag="xnew", name=f"xnew_{r}_{t}")
                if r == 1:
                    nc.vector.tensor_copy(xnew[:], ps[:])
                else:
                    xprev_src = x0s if r == 2 else xs1
                    xp = fp.tile([128, D], f32, tag="xp", name=f"xp_{r}_{t}")
                    nc.sync.dma_start(
                        out=xp[:], in_=xprev_src[t * RT:(t + 1) * RT, :])
                    nc.vector.scalar_tensor_tensor(
                        xnew[:], ps[:], 2.0, xp[:],
                        op0=mybir.AluOpType.mult,
                        op1=mybir.AluOpType.subtract)
                if r == 1:
                    nc.sync.dma_start(
                        out=xs1[t * RT:(t + 1) * RT, :], in_=xnew[:])
                if r <= 2:
                    xbt = fp.tile([128, D], bf16, tag="xbt",
                                  name=f"xbt_{r}_{t}")
                    nc.any.tensor_copy(xbt[:], xnew[:])
                    nc.sync.dma_start(
                        out=xb[r - 1][t * RT:(t + 1) * RT, :], in_=xbt[:])
                # feature-major copy for the einsum: feature row 2p+j is
                # partition p of plane j
                pab = pabp.tile([128, 256], f32, tag="pab",
                                name=f"pab_{r}_{t}")
                xe = xnew[:].rearrange("v (f two) -> v two f", two=2)
                nc.tensor.transpose(pab[:, 0:128], xe[:, 0, :], ident_t[:])
                nc.tensor.transpose(pab[:, 128:256], xe[:, 1, :], ident_t[:])
                xtp = fp.tile([128, 2, 128], bf16, tag="xtp",
                              name=f"xtp_{r}_{t}")
                nc.any.tensor_copy(
                    xtp[:].rearrange("p two v -> p (two v)"), pab[:])
                nc.sync.dma_start(
                    out=xt[r - 1].rearrange(
                        "(f two) v -> f two v", two=2)[:, :, t * RT:(t + 1) * RT],
                    in_=xtp[:])

            def allgather(r, part):
                lo = 0 if part == 0 else ag_bounds[part - 1] * RT
                hi = ag_bounds[part] * RT
                if ag_mode == "collective":
                    nc.gpsimd.collective_compute(
                        "AllGather", mybir.AluOpType.bypass,
                        replica_groups=AG_GROUPS,
                        ins=[xb[r - 1][lo:hi, :]],
                        outs=[xf[r - 1].rearrange(
                            "(c v) d -> c v d", c=NC)[:, lo:hi, :]])
                else:  # single-core timing stand-in: same DRAM traffic
                    for c in range(NC):
                        nc.sync.dma_start(
                            out=xf[r - 1][c * VSP + lo:c * VSP + hi, :],
                            in_=xb[r - 1][lo:hi, :])

            def spmm_round(r):
                src = xg0 if r == 1 else xf[r - 2]
                live_ps = {}
                fin_done = 0
                ag_next = 0
                for step in prog["steps"]:
                    ch = step["cc"]
                    n = step["n"]
                    io = step["idx_off"]
                    zq = zqp.tile([128, n // 128, D], bf16, tag="zq",
                                  name=f"zq_{r}_{io}")
                    nc.gpsimd.dma_gather(
                        zq[:], src[ch * CHUNK:(ch + 1) * CHUNK, :],
                        gidx_t[:, io // 16:(io + n) // 16],
                        num_idxs=n, num_idxs_reg=n, elem_size=D,
                        queue_num=qn[0] % NQ)
                    qn[0] += 1
                    for j, grp in enumerate(step["groups"]):
                        g, t = grp["g"], grp["rt"]
                        if grp["start"]:
                            live_ps[t] = pp.tile(
                                [128, D], f32, tag=f"ps{t % cfg['TBLK']}",
                                name=f"ps_{r}_{t}")
                        st = stp.tile([128, RT], bf16, tag="st",
                                      name=f"st_{r}_{g}")
                        nc.any.tensor_scalar(
                            st[:], iota_t[:],
                            meta_t[:, g, 0:1], meta_t[:, g, 1:2],
                            op0=mybir.AluOpType.is_equal,
                            op1=mybir.AluOpType.mult)
                        nc.tensor.matmul(
                            live_ps[t][:], st[:], zq[:, j, :],
                            start=grp["start"], stop=grp["stop"])
                    for t in step["fin"]:
                        finalize(r, t, live_ps.pop(t))
                        fin_done += 1
                        if (r <= 2 and ag_next < len(ag_bounds)
                                and fin_done == ag_bounds[ag_next]):
                            allgather(r, ag_next)
                            ag_next += 1
                assert fin_done == NT and not live_ps

            for r in (1, 2, 3):
                spmm_round(r)

        # einsum: outT[b][o, v] = sum_k W_k^T @ T_k^T[b-rows, v] + bias
        with (
            tc.tile_pool(name="ew", bufs=1) as ewp,
            tc.tile_pool(name="erhs", bufs=3) as erp,
            tc.tile_pool(name="eout", bufs=3) as eop,
            tc.tile_pool(name="epsum", bufs=1, space="PSUM") as epp,
        ):
            w0_t = ewp.tile([cfg["FIN"], FOUT], f32)
            wb_t = ewp.tile([cfg["FIN"], 3, FOUT], bf16)
            bias_t = ewp.tile([FOUT, 1], f32)
            nc.sync.dma_start(out=w0_t[:], in_=w0_d[:])
            nc.sync.dma_start(out=wb_t[:], in_=wb_d[:])
            nc.sync.dma_start(out=bias_t[:], in_=bias_d[:])
            VC = 512
            nvc = (VSP + VC - 1) // VC
            for v in range(nvc):
                v0 = v * VC
                vn = min(VC, VSP - v0)
                for bb in range(B):
                    f0 = bb * cfg["FIN"]
                    r0 = erp.tile([cfg["FIN"], VC], f32, tag="r0",
                                  name=f"r0_{v}_{bb}")
                    nc.sync.dma_start(
                        out=r0[:, :vn], in_=x0t[f0:f0 + cfg["FIN"], v0:v0 + vn])
                    rk = {}
                    for k in (1, 2, 3):
                        rt_ = erp.tile([cfg["FIN"], VC], bf16, tag=f"rk{k}",
                                       name=f"rk_{v}_{bb}_{k}")
                        nc.sync.dma_start(
                            out=rt_[:, :vn],
                            in_=xt[k - 1][f0:f0 + cfg["FIN"], v0:v0 + vn])
                        rk[k] = rt_
                    ops = epp.tile([FOUT, VC], f32, tag=f"eps{bb % 4}",
                                   name=f"eps_{v}_{bb}")
                    nc.tensor.matmul(ops[:, :vn], w0_t[:], r0[:, :vn],
                                     start=True, stop=False)
                    for k in (1, 2, 3):
                        nc.tensor.matmul(ops[:, :vn], wb_t[:, k - 1, :],
                                         rk[k][:, :vn],
                                         start=False, stop=(k == 3))
                    ot = eop.tile([FOUT, VC], f32, tag="ot",
                                  name=f"ot_{v}_{bb}")
                    nc.vector.tensor_scalar(
                        ot[:, :vn], ops[:, :vn], bias_t[:], None,
                        op0=mybir.AluOpType.add)
                    nc.sync.dma_start(out=outT[bb][:, v0:v0 + vn], in_=ot[:, :vn])

    nc.compile()
    return nc


def _host_prep(x, weight, bias, lap_vals, lap_rows, lap_cols, cfg):
    NC, VS, VSP, VG, D = cfg["NC"], cfg["VS"], cfg["VSP"], cfg["VG"], cfg["D"]
    V = cfg["V"]
    x = np.asarray(x, dtype=np.float32)
    x0 = np.ascontiguousarray(x.transpose(1, 0, 2).reshape(V, D))  # [V, B*FIN]

    prog, per_core = preprocess(lap_rows, lap_cols, lap_vals, cfg)

    x0p = np.zeros((VG, D), dtype=np.float32)
    for c in range(NC):
        x0p[c * VSP:c * VSP + VS] = x0[c * VS:(c + 1) * VS]
    xg0 = x0p.astype(BF16)

    iota = np.tile(np.arange(cfg["RT"], dtype=np.float32).astype(BF16), (128, 1))
    ident = np.eye(128, dtype=np.float32)
    weight = np.asarray(weight, dtype=np.float32)
    w0 = weight[0]
    wb = np.zeros((cfg["FIN"], 3, cfg["FOUT"]), dtype=BF16)
    for k in (1, 2, 3):
        wb[:, k - 1] = weight[k].astype(BF16)
    bias_in = np.asarray(bias, dtype=np.float32).reshape(cfg["FOUT"], 1)

    in_maps = []
    for c in range(NC):
        x0sh = x0p[c * VSP:(c + 1) * VSP]
        in_maps.append({
            "xg0": xg0,
            "x0s": np.ascontiguousarray(x0sh),
            "x0t": np.ascontiguousarray(x0sh.T),
            "gidx": per_core[c]["gidx"],
            "meta": per_core[c]["meta"],
            "iota": iota,
            "ident": ident,
            "w0": w0,
            "wb": wb,
            "biasin": bias_in,
        })
    return prog, in_maps


def _assemble(results, cfg):
    NC, VS, VSP, B, FOUT, V = (cfg["NC"], cfg["VS"], cfg["VSP"], cfg["B"],
                               cfg["FOUT"], cfg["V"])
    out = np.empty((B, V, FOUT), dtype=np.float32)
    for c in range(NC):
        oT = np.asarray(results[c]["outT"]).reshape(B, FOUT, VSP)
        out[:, c * VS:(c + 1) * VS, :] = oT.transpose(0, 2, 1)[:, :VS, :]
    return out


def run(x, weight, bias, lap_vals, lap_rows, lap_cols, trace=False):
    """Returns (output, BassKernelResults)."""
    from concourse import bass_utils

    cfg = make_cfg()
    prog, in_maps = _host_prep(x, weight, bias, lap_vals, lap_rows, lap_cols, cfg)
    nc = build_nc(cfg, prog)
    res = bass_utils.run_bass_kernel_spmd(nc, in_maps, list(range(cfg["NC"])),
                                          trace=trace)
    return _assemble(res.results, cfg), res


def kernel(x, weight, bias, lap_vals, lap_rows, lap_cols):
    out, _ = run(x, weight, bias, lap_vals, lap_rows, lap_cols)
    return out


# revision 9
# speedup vs baseline: 9.5793x; 1.1425x over previous
"""ChebConv (K=4) GNN layer on 8 Trainium2 NeuronCores.

Strategy (sharding_hint: row-partition of the Laplacian + replicated weight):
  - Nodes V row-sharded across 8 cores (VS rows each, padded to VSP=NT*128).
  - Each core owns the edges whose destination row lives in its shard.
  - The current poly y lives as a full node-major bf16 replica [VG, 256] in
    DRAM (AllGather of the 8 shards per spmm round). Edge gathers use
    GPSIMD-issued SWDGE dma_gather: each index pulls one node's 512B feature
    row from HBM straight into SBUF at partition i%128, slot i//128 — i.e.
    a [128 edges, 256 feat] matmul-ready block, no transposes.
  - Edges are ordered tile-major: for each block of TBLK row tiles, for each
    of the NCC=4 node chunks (int16 gather index range), the (tile, chunk)
    edge cells. Segment-sum is a one-hot matmul S[e, r]=val*(rloc==r)
    accumulated in a per-tile PSUM bank across the whole stream.
  - A row tile finalizes right after its last matmul: Chebyshev recurrence
    on the vector engine, bf16 shard write (AllGather input), and a pair of
    PE transposes producing the feature-major copy the final einsum needs.
    The AllGather is split in two halves so the first fires mid-round.
  - Final einsum contracts T_k with the replicated weight on the PE.

The instruction stream is identical on all cores (SPMD): per-(tile, chunk)
edge-cell sizes are padded to the max across cores, so only the index /
one-hot data differs per core.
"""

import sys

import numpy as np

sys.path.insert(0, "/opt/trn_rl_repo")

import ml_dtypes  # noqa: E402

BF16 = ml_dtypes.bfloat16


def make_cfg(V=100000, E=1600000, B=4, FIN=64, FOUT=64, NC=8, RT=128,
             NCC=4, TBLK=4, CALLMAX=1024, NQ=4, SCRATCH=65536):
    VS = V // NC
    assert VS * NC == V
    VSP = ((VS + RT - 1) // RT) * RT
    NT = VSP // RT
    VG = VSP * NC
    CHUNK = VG // NCC          # nodes per gather chunk (int16 idx range)
    assert CHUNK * NCC == VG
    assert CHUNK <= 32768      # int16 dma_gather row index limit
    D = B * FIN
    assert CALLMAX <= SCRATCH // 16 // NQ  # per-queue SWDGE ring capacity
    return dict(V=V, E=E, B=B, FIN=FIN, FOUT=FOUT, NC=NC, RT=RT,
                CHUNK=CHUNK, CALLMAX=CALLMAX, VS=VS, VSP=VSP, NT=NT, VG=VG,
                NCC=NCC, TBLK=TBLK, D=D, NQ=NQ, SCRATCH=SCRATCH)


def _wrap16(idx, npart=128):
    """Pack an idx list (len n, multiple of 16) into the SWDGE gather layout:
    idx i at partition i%16, slot i//16, replicated to all 16-partition
    groups."""
    n = idx.shape[0]
    w = idx.reshape(n // 16, 16).T  # [16, n/16]
    return np.tile(w, (npart // 16, 1))


def preprocess(rows, cols, vals, cfg):
    """Build the static SPMD schedule + per-core index/one-hot data.

    Returns (prog, per_core): prog is core-independent structure;
    per_core[c] has 'gidx' [128, NIDX/16] int16 and 'meta' [128, NG, 2] f32.
    """
    NC, VS, VSP, RT, NT = cfg["NC"], cfg["VS"], cfg["VSP"], cfg["RT"], cfg["NT"]
    CHUNK, CALLMAX, NCC, TBLK = (cfg["CHUNK"], cfg["CALLMAX"], cfg["NCC"],
                                 cfg["TBLK"])

    rows = np.asarray(rows, dtype=np.int64)
    cols = np.asarray(cols, dtype=np.int64)
    vals = np.asarray(vals, dtype=np.float32)

    owner = rows // VS
    lr = rows - owner * VS
    rt = lr // RT
    rloc = lr - rt * RT
    gc = (cols // VS) * VSP + (cols % VS)   # padded-global replica row
    cc = gc // CHUNK
    ci = (gc - cc * CHUNK).astype(np.int64)  # row idx within chunk

    # per-core edge cells keyed by (rt, cc)
    cell_of = rt * NCC + cc
    ncells = NT * NCC
    counts = np.zeros((NC, ncells), dtype=np.int64)
    for c in range(NC):
        m = owner == c
        counts[c] = np.bincount(cell_of[m], minlength=ncells)
    mx = counts.max(axis=0)
    mpad = ((mx + RT - 1) // RT) * RT  # padded cell size, common to all cores
    mpad2 = mpad.reshape(NT, NCC)
    # every tile needs at least one group so its PSUM accumulator exists
    for t in range(NT):
        if mpad2[t].sum() == 0:
            mpad2[t, 0] = RT

    per_core_cells = []
    for c in range(NC):
        m = owner == c
        order = np.argsort(cell_of[m], kind="stable")
        e_ci = ci[m][order]
        e_rloc = rloc[m][order]
        e_val = vals[m][order]
        e_cell = cell_of[m][order]
        starts = np.searchsorted(e_cell, np.arange(ncells))
        ends = np.searchsorted(e_cell, np.arange(ncells) + 1)
        per_core_cells.append((e_ci, e_rloc, e_val, starts, ends))

    NIDX = int(mpad2.sum())
    NG = NIDX // RT

    gidx = [np.zeros(NIDX, dtype=np.int16) for _ in range(NC)]
    gridx = [np.zeros((NG, RT), dtype=np.float32) for _ in range(NC)]
    gval = [np.zeros((NG, RT), dtype=np.float32) for _ in range(NC)]

    # per-tile first/last group for start/stop flags
    tile_ngroups = (mpad2 // RT).sum(axis=1)
    steps = []
    pos = 0
    gpos = 0
    for tb in range(0, NT, TBLK):
        tiles = list(range(tb, min(tb + TBLK, NT)))
        remaining = {t: int(tile_ngroups[t]) for t in tiles}
        started = set()
        for ch in range(NCC):
            # groups of cells (t, ch) for t in tiles, in tile order
            seg_groups = []
            for t in tiles:
                n = int(mpad2[t, ch])
                if n == 0:
                    continue
                for c in range(NC):
                    e_ci, e_rloc, e_val, starts_, ends_ = per_core_cells[c]
                    s_, e_ = starts_[t * NCC + ch], ends_[t * NCC + ch]
                    k = e_ - s_
                    gidx[c][pos:pos + k] = e_ci[s_:e_].astype(np.int16)
                    gr = gridx[c][gpos:gpos + n // RT].reshape(-1)
                    gv = gval[c][gpos:gpos + n // RT].reshape(-1)
                    gr[:k] = e_rloc[s_:e_].astype(np.float32)
                    gv[:k] = e_val[s_:e_].astype(np.float32)
                ngr = n // RT
                for j in range(ngr):
                    st = t not in started
                    started.add(t)
                    remaining[t] -= 1
                    seg_groups.append({"g": gpos + j, "rt": t, "start": st,
                                       "stop": remaining[t] == 0})
                pos += n
                gpos += ngr
            # pack groups into calls of <= CALLMAX idxs
            gi = 0
            off0 = pos - len(seg_groups) * RT
            while gi < len(seg_groups):
                take = min(CALLMAX // RT, len(seg_groups) - gi)
                grps = seg_groups[gi:gi + take]
                fin = [g["rt"] for g in grps if g["stop"]]
                steps.append({"cc": ch, "idx_off": off0 + gi * RT,
                              "n": take * RT, "groups": grps, "fin": fin})
                gi += take
    assert pos == NIDX and gpos == NG

    per_core = []
    for c in range(NC):
        meta = np.zeros((128, NG, 2), dtype=np.float32)
        meta[:, :, 0] = gridx[c].T
        meta[:, :, 1] = gval[c].T
        per_core.append({"gidx": _wrap16(gidx[c]), "meta": meta})

    prog = {"NIDX": NIDX, "NG": NG, "steps": steps}
    return prog, per_core


def build_nc(cfg, prog, ag_mode="collective", ag_split=1):
    import concourse.bacc as bacc
    import concourse.mybir as mybir
    import concourse.tile as tile

    NC, VSP, VG, D, RT, NT = (cfg["NC"], cfg["VSP"], cfg["VG"], cfg["D"],
                              cfg["RT"], cfg["NT"])
    CHUNK, B, FOUT, NQ = cfg["CHUNK"], cfg["B"], cfg["FOUT"], cfg["NQ"]
    NIDX, NG = prog["NIDX"], prog["NG"]
    f32, bf16, i16 = mybir.dt.float32, mybir.dt.bfloat16, mybir.dt.int16
    AG_GROUPS = [list(range(NC))]
    # AllGather split boundaries (by row tile)
    ag_bounds = [NT * (i + 1) // ag_split for i in range(ag_split)]

    nc = bacc.Bacc("TRN2", target_bir_lowering=False, debug=False,
                   num_devices=NC, num_swdge_queues=NQ,
                   dynamic_dma_scratch_size=cfg["SCRATCH"])

    # inputs
    xg0 = nc.dram_tensor("xg0", [VG, D], bf16, kind="ExternalInput")
    x0s = nc.dram_tensor("x0s", [VSP, D], f32, kind="ExternalInput")
    x0t = nc.dram_tensor("x0t", [D, VSP], f32, kind="ExternalInput")
    gidx_d = nc.dram_tensor("gidx", [128, NIDX // 16], i16, kind="ExternalInput")
    meta_d = nc.dram_tensor("meta", [128, NG, 2], f32, kind="ExternalInput")
    iota_d = nc.dram_tensor("iota", [128, RT], bf16, kind="ExternalInput")
    ident_d = nc.dram_tensor("ident", [128, 128], f32, kind="ExternalInput")
    w0_d = nc.dram_tensor("w0", [cfg["FIN"], FOUT], f32, kind="ExternalInput")
    wb_d = nc.dram_tensor("wb", [cfg["FIN"], 3, FOUT], bf16, kind="ExternalInput")
    bias_d = nc.dram_tensor("biasin", [FOUT, 1], f32, kind="ExternalInput")

    # outputs
    outT = nc.dram_tensor("outT", [B, FOUT, VSP], f32, kind="ExternalOutput")

    # internal DRAM
    xb = [nc.dram_tensor(f"xb{k}", [VSP, D], bf16) for k in (1, 2)]
    xf = [nc.dram_tensor(f"xf{k}", [VG, D], bf16, addr_space="Shared")
          for k in (1, 2)]
    xt = [nc.dram_tensor(f"xt{k}", [D, VSP], bf16) for k in (1, 2, 3)]
    xs1 = nc.dram_tensor("xs1", [VSP, D], f32)

    qn = [0]

    with tile.TileContext(nc) as tc:
        with (
            tc.tile_pool(name="static", bufs=1) as sp,
            tc.tile_pool(name="zqp", bufs=4) as zqp,
            tc.tile_pool(name="stile", bufs=4) as stp,
            tc.tile_pool(name="fin", bufs=3) as fp,
            tc.tile_pool(name="psum", bufs=1, space="PSUM") as pp,
            tc.tile_pool(name="pab", bufs=2, space="PSUM") as pabp,
        ):
            gidx_t = sp.tile([128, NIDX // 16], i16)
            meta_t = sp.tile([128, NG, 2], f32)
            iota_t = sp.tile([128, RT], bf16)
            ident_t = sp.tile([128, 128], f32)
            nc.sync.dma_start(out=gidx_t[:], in_=gidx_d[:])
            nc.sync.dma_start(out=meta_t[:], in_=meta_d[:])
            nc.sync.dma_start(out=iota_t[:], in_=iota_d[:])
            nc.sync.dma_start(out=ident_t[:], in_=ident_d[:])

            def finalize(r, t, ps):
                xnew = fp.tile([128, D], f32, tag="xnew", name=f"xnew_{r}_{t}")
                if r == 1:
                    nc.vector.tensor_copy(xnew[:], ps[:])
                else:
                    xprev_src = x0s if r == 2 else xs1
                    xp = fp.tile([128, D], f32, tag="xp", name=f"xp_{r}_{t}")
                    nc.sync.dma_start(
                        out=xp[:], in_=xprev_src[t * RT:(t + 1) * RT, :])
                    nc.vector.scalar_tensor_tensor(
                        xnew[:], ps[:], 2.0, xp[:],
                        op0=mybir.AluOpType.mult,
                        op1=mybir.AluOpType.subtract)
                if r == 1:
                    nc.sync.dma_start(
                        out=xs1[t * RT:(t + 1) * RT, :], in_=xnew[:])
                if r <= 2:
                    xbt = fp.tile([128, D], bf16, tag="xbt",
                                  name=f"xbt_{r}_{t}")
                    nc.any.tensor_copy(xbt[:], xnew[:])
                    nc.sync.dma_start(
                        out=xb[r - 1][t * RT:(t + 1) * RT, :], in_=xbt[:])
                # feature-major copy for the einsum: feature row 2p+j is
                # partition p of plane j
                pab = pabp.tile([128, 256], f32, tag="pab",
                                name=f"pab_{r}_{t}")
                xe = xnew[:].rearrange("v (f two) -> v two f", two=2)
                nc.tensor.transpose(pab[:, 0:128], xe[:, 0, :], ident_t[:])
                nc.tensor.transpose(pab[:, 128:256], xe[:, 1, :], ident_t[:])
                xtp = fp.tile([128, 2, 128], bf16, tag="xtp",
                              name=f"xtp_{r}_{t}")
                nc.any.tensor_copy(
                    xtp[:].rearrange("p two v -> p (two v)"), pab[:])
                nc.sync.dma_start(
                    out=xt[r - 1].rearrange(
                        "(f two) v -> f two v", two=2)[:, :, t * RT:(t + 1) * RT],
                    in_=xtp[:])

            def allgather(r, part):
                lo = 0 if part == 0 else ag_bounds[part - 1] * RT
                hi = ag_bounds[part] * RT
                if ag_mode == "collective":
                    nc.gpsimd.collective_compute(
                        "AllGather", mybir.AluOpType.bypass,
                        replica_groups=AG_GROUPS,
                        ins=[xb[r - 1][lo:hi, :]],
                        outs=[xf[r - 1].rearrange(
                            "(c v) d -> c v d", c=NC)[:, lo:hi, :]])
                else:  # single-core timing stand-in: same DRAM traffic
                    for c in range(NC):
                        nc.sync.dma_start(
                            out=xf[r - 1][c * VSP + lo:c * VSP + hi, :],
                            in_=xb[r - 1][lo:hi, :])

            def spmm_round(r):
                src = xg0 if r == 1 else xf[r - 2]
                live_ps = {}
                fin_done = 0
                ag_next = 0
                for step in prog["steps"]:
                    ch = step["cc"]
                    n = step["n"]
                    io = step["idx_off"]
                    zq = zqp.tile([128, n // 128, D], bf16, tag="zq",
                                  name=f"zq_{r}_{io}")
                    nc.gpsimd.dma_gather(
                        zq[:], src[ch * CHUNK:(ch + 1) * CHUNK, :],
                        gidx_t[:, io // 16:(io + n) // 16],
                        num_idxs=n, num_idxs_reg=n, elem_size=D,
                        queue_num=qn[0] % NQ)
                    qn[0] += 1
                    for j, grp in enumerate(step["groups"]):
                        g, t = grp["g"], grp["rt"]
                        if grp["start"]:
                            live_ps[t] = pp.tile(
                                [128, D], f32, tag=f"ps{t % cfg['TBLK']}",
                                name=f"ps_{r}_{t}")
                        st = stp.tile([128, RT], bf16, tag="st",
                                      name=f"st_{r}_{g}")
                        nc.any.tensor_scalar(
                            st[:], iota_t[:],
                            meta_t[:, g, 0:1], meta_t[:, g, 1:2],
                            op0=mybir.AluOpType.is_equal,
                            op1=mybir.AluOpType.mult)
                        nc.tensor.matmul(
                            live_ps[t][:], st[:], zq[:, j, :],
                            start=grp["start"], stop=grp["stop"])
                    for t in step["fin"]:
                        finalize(r, t, live_ps.pop(t))
                        fin_done += 1
                        if (r <= 2 and ag_next < len(ag_bounds)
                                and fin_done == ag_bounds[ag_next]):
                            allgather(r, ag_next)
                            ag_next += 1
                assert fin_done == NT and not live_ps

            for r in (1, 2, 3):
                spmm_round(r)

        # einsum: outT[b][o, v] = sum_k W_k^T @ T_k^T[b-rows, v] + bias
        with (
            tc.tile_pool(name="ew", bufs=1) as ewp,
            tc.tile_pool(name="erhs", bufs=3) as erp,
            tc.tile_pool(name="eout", bufs=3) as eop,
            tc.tile_pool(name="epsum", bufs=1, space="PSUM") as epp,
        ):
            w0_t = ewp.tile([cfg["FIN"], FOUT], f32)
            wb_t = ewp.tile([cfg["FIN"], 3, FOUT], bf16)
            bias_t = ewp.tile([FOUT, 1], f32)
            nc.sync.dma_start(out=w0_t[:], in_=w0_d[:])
            nc.sync.dma_start(out=wb_t[:], in_=wb_d[:])
            nc.sync.dma_start(out=bias_t[:], in_=bias_d[:])
            VC = 512
            nvc = (VSP + VC - 1) // VC
            for v in range(nvc):
                v0 = v * VC
                vn = min(VC, VSP - v0)
                for bb in range(B):
                    f0 = bb * cfg["FIN"]
                    r0 = erp.tile([cfg["FIN"], VC], f32, tag="r0",
                                  name=f"r0_{v}_{bb}")
                    nc.sync.dma_start(
                        out=r0[:, :vn], in_=x0t[f0:f0 + cfg["FIN"], v0:v0 + vn])
                    rk = {}
                    for k in (1, 2, 3):
                        rt_ = erp.tile([cfg["FIN"], VC], bf16, tag=f"rk{k}",
                                       name=f"rk_{v}_{bb}_{k}")
                        nc.sync.dma_start(
                            out=rt_[:, :vn],
                            in_=xt[k - 1][f0:f0 + cfg["FIN"], v0:v0 + vn])
                        rk[k] = rt_
                    ops = epp.tile([FOUT, VC], f32, tag=f"eps{bb % 4}",
                                   name=f"eps_{v}_{bb}")
                    nc.tensor.matmul(ops[:, :vn], w0_t[:], r0[:, :vn],
                                     start=True, stop=False)
                    for k in (1, 2, 3):
                        nc.tensor.matmul(ops[:, :vn], wb_t[:, k - 1, :],
                                         rk[k][:, :vn],
                                         start=False, stop=(k == 3))
                    ot = eop.tile([FOUT, VC], f32, tag="ot",
                                  name=f"ot_{v}_{bb}")
                    nc.vector.tensor_scalar(
                        ot[:, :vn], ops[:, :vn], bias_t[:], None,
                        op0=mybir.AluOpType.add)
                    nc.sync.dma_start(out=outT[bb][:, v0:v0 + vn], in_=ot[:, :vn])

    nc.compile()
    return nc


def _host_prep(x, weight, bias, lap_vals, lap_rows, lap_cols, cfg):
    NC, VS, VSP, VG, D = cfg["NC"], cfg["VS"], cfg["VSP"], cfg["VG"], cfg["D"]
    V = cfg["V"]
    x = np.asarray(x, dtype=np.float32)
    x0 = np.ascontiguousarray(x.transpose(1, 0, 2).reshape(V, D))  # [V, B*FIN]

    prog, per_core = preprocess(lap_rows, lap_cols, lap_vals, cfg)

    x0p = np.zeros((VG, D), dtype=np.float32)
    for c in range(NC):
        x0p[c * VSP:c * VSP + VS] = x0[c * VS:(c + 1) * VS]
    xg0 = x0p.astype(BF16)

    iota = np.tile(np.arange(cfg["RT"], dtype=np.float32).astype(BF16), (128, 1))
    ident = np.eye(128, dtype=np.float32)
    weight = np.asarray(weight, dtype=np.float32)
    w0 = weight[0]
    wb = np.zeros((cfg["FIN"], 3, cfg["FOUT"]), dtype=BF16)
    for k in (1, 2, 3):
        wb[:, k - 1] = weight[k].astype(BF16)
    bias_in = np.asarray(bias, dtype=np.float32).reshape(cfg["FOUT"], 1)

    in_maps = []
    for c in range(NC):
        x0sh = x0p[c * VSP:(c + 1) * VSP]
        in_maps.append({
            "xg0": xg0,
            "x0s": np.ascontiguousarray(x0sh),
            "x0t": np.ascontiguousarray(x0sh.T),
            "gidx": per_core[c]["gidx"],
            "meta": per_core[c]["meta"],
            "iota": iota,
            "ident": ident,
            "w0": w0,
            "wb": wb,
            "biasin": bias_in,
        })
    return prog, in_maps


def _assemble(results, cfg):
    NC, VS, VSP, B, FOUT, V = (cfg["NC"], cfg["VS"], cfg["VSP"], cfg["B"],
                               cfg["FOUT"], cfg["V"])
    out = np.empty((B, V, FOUT), dtype=np.float32)
    for c in range(NC):
        oT = np.asarray(results[c]["outT"]).reshape(B, FOUT, VSP)
        out[:, c * VS:(c + 1) * VS, :] = oT.transpose(0, 2, 1)[:, :VS, :]
    return out


def run(x, weight, bias, lap_vals, lap_rows, lap_cols, trace=False):
    """Returns (output, BassKernelResults)."""
    from concourse import bass_utils

    cfg = make_cfg()
    prog, in_maps = _host_prep(x, weight, bias, lap_vals, lap_rows, lap_cols, cfg)
    nc = build_nc(cfg, prog)
    res = bass_utils.run_bass_kernel_spmd(nc, in_maps, list(range(cfg["NC"])),
                                          trace=trace)
    return _assemble(res.results, cfg), res


def kernel(x, weight, bias, lap_vals, lap_rows, lap_cols):
    out, _ = run(x, weight, bias, lap_vals, lap_rows, lap_cols)
    return out


# revision 19
# speedup vs baseline: 10.6680x; 1.1137x over previous
"""ChebConv (K=4) GNN layer on 8 Trainium2 NeuronCores.

Strategy (sharding_hint: row-partition of the Laplacian + replicated weight):
  - Nodes V row-sharded across 8 cores (VS rows each, padded to VSP=NT*128).
  - Each core owns the edges whose destination row lives in its shard.
  - The current poly y lives as a full node-major bf16 replica [VG, 256] in
    DRAM (AllGather of the 8 shards per spmm round). Edge gathers use
    GPSIMD-issued SWDGE dma_gather: each index pulls one node's 512B feature
    row from HBM straight into SBUF at partition i%128, slot i//128 — i.e.
    a [128 edges, 256 feat] matmul-ready block, no transposes.
  - Edges are ordered tile-major: for each block of TBLK row tiles, for each
    of the NCC=4 node chunks (int16 gather index range), the (tile, chunk)
    edge cells. Segment-sum is a one-hot matmul S[e, r]=val*(rloc==r)
    accumulated in a per-tile PSUM bank across the whole stream.
  - A row tile finalizes right after its last matmul: Chebyshev recurrence
    on the vector engine, bf16 shard write (AllGather input), and a pair of
    PE transposes producing the feature-major copy the final einsum needs.
    The AllGather is split in two halves so the first fires mid-round.
  - Final einsum contracts T_k with the replicated weight on the PE.

The instruction stream is identical on all cores (SPMD): per-(tile, chunk)
edge-cell sizes are padded to the max across cores, so only the index /
one-hot data differs per core.
"""

import sys

import numpy as np

sys.path.insert(0, "/opt/trn_rl_repo")

import ml_dtypes  # noqa: E402

BF16 = ml_dtypes.bfloat16


def make_cfg(V=100000, E=1600000, B=4, FIN=64, FOUT=64, NC=8, RT=128,
             NCC=4, TBLK=4, CALLMAX=1024, NQ=4, SCRATCH=65536):
    VS = V // NC
    assert VS * NC == V
    VSP = ((VS + RT - 1) // RT) * RT
    NT = VSP // RT
    VG = VSP * NC
    CHUNK = VG // NCC          # nodes per gather chunk (int16 idx range)
    assert CHUNK * NCC == VG
    assert CHUNK <= 32768      # int16 dma_gather row index limit
    D = B * FIN
    assert CALLMAX <= SCRATCH // 16 // NQ  # per-queue SWDGE ring capacity
    return dict(V=V, E=E, B=B, FIN=FIN, FOUT=FOUT, NC=NC, RT=RT,
                CHUNK=CHUNK, CALLMAX=CALLMAX, VS=VS, VSP=VSP, NT=NT, VG=VG,
                NCC=NCC, TBLK=TBLK, D=D, NQ=NQ, SCRATCH=SCRATCH)


def _wrap16(idx, npart=128):
    """Pack an idx list (len n, multiple of 16) into the SWDGE gather layout:
    idx i at partition i%16, slot i//16, replicated to all 16-partition
    groups."""
    n = idx.shape[0]
    w = idx.reshape(n // 16, 16).T  # [16, n/16]
    return np.tile(w, (npart // 16, 1))


def preprocess(rows, cols, vals, cfg):
    """Build the static SPMD schedule + per-core index/one-hot data.

    Returns (prog, per_core): prog is core-independent structure;
    per_core[c] has 'gidx' [128, NIDX/16] int16 and 'meta' [128, NG, 2] f32.
    """
    NC, VS, VSP, RT, NT = cfg["NC"], cfg["VS"], cfg["VSP"], cfg["RT"], cfg["NT"]
    CHUNK, CALLMAX, NCC, TBLK = (cfg["CHUNK"], cfg["CALLMAX"], cfg["NCC"],
                                 cfg["TBLK"])

    rows = np.asarray(rows, dtype=np.int64)
    cols = np.asarray(cols, dtype=np.int64)
    vals = np.asarray(vals, dtype=np.float32)

    owner = rows // VS
    lr = rows - owner * VS
    rt = lr // RT
    rloc = lr - rt * RT
    gc = (cols // VS) * VSP + (cols % VS)   # padded-global replica row
    cc = gc // CHUNK
    ci = (gc - cc * CHUNK).astype(np.int64)  # row idx within chunk

    # per-core edge cells keyed by (rt, cc)
    cell_of = rt * NCC + cc
    ncells = NT * NCC
    counts = np.zeros((NC, ncells), dtype=np.int64)
    for c in range(NC):
        m = owner == c
        counts[c] = np.bincount(cell_of[m], minlength=ncells)
    mx = counts.max(axis=0)
    mpad = ((mx + RT - 1) // RT) * RT  # padded cell size, common to all cores
    mpad2 = mpad.reshape(NT, NCC)
    # every tile needs at least one group so its PSUM accumulator exists
    for t in range(NT):
        if mpad2[t].sum() == 0:
            mpad2[t, 0] = RT

    per_core_cells = []
    for c in range(NC):
        m = owner == c
        order = np.argsort(cell_of[m], kind="stable")
        e_ci = ci[m][order]
        e_rloc = rloc[m][order]
        e_val = vals[m][order]
        e_cell = cell_of[m][order]
        starts = np.searchsorted(e_cell, np.arange(ncells))
        ends = np.searchsorted(e_cell, np.arange(ncells) + 1)
        per_core_cells.append((e_ci, e_rloc, e_val, starts, ends))

    NIDX = int(mpad2.sum())
    NG = NIDX // RT

    gidx = [np.zeros(NIDX, dtype=np.int16) for _ in range(NC)]
    gridx = [np.zeros((NG, RT), dtype=np.float32) for _ in range(NC)]
    gval = [np.zeros((NG, RT), dtype=np.float32) for _ in range(NC)]

    # per-tile first/last group for start/stop flags
    tile_ngroups = (mpad2 // RT).sum(axis=1)
    steps = []
    pos = 0
    gpos = 0
    for tb in range(0, NT, TBLK):
        tiles = list(range(tb, min(tb + TBLK, NT)))
        remaining = {t: int(tile_ngroups[t]) for t in tiles}
        started = set()
        for ch in range(NCC):
            # groups of cells (t, ch) for t in tiles, in tile order
            seg_groups = []
            for t in tiles:
                n = int(mpad2[t, ch])
                if n == 0:
                    continue
                for c in range(NC):
                    e_ci, e_rloc, e_val, starts_, ends_ = per_core_cells[c]
                    s_, e_ = starts_[t * NCC + ch], ends_[t * NCC + ch]
                    k = e_ - s_
                    gidx[c][pos:pos + k] = e_ci[s_:e_].astype(np.int16)
                    gr = gridx[c][gpos:gpos + n // RT].reshape(-1)
                    gv = gval[c][gpos:gpos + n // RT].reshape(-1)
                    gr[:k] = e_rloc[s_:e_].astype(np.float32)
                    gv[:k] = e_val[s_:e_].astype(np.float32)
                ngr = n // RT
                for j in range(ngr):
                    st = t not in started
                    started.add(t)
                    remaining[t] -= 1
                    seg_groups.append({"g": gpos + j, "rt": t, "start": st,
                                       "stop": remaining[t] == 0})
                pos += n
                gpos += ngr
            # pack groups into calls of <= CALLMAX idxs
            gi = 0
            off0 = pos - len(seg_groups) * RT
            while gi < len(seg_groups):
                take = min(CALLMAX // RT, len(seg_groups) - gi)
                grps = seg_groups[gi:gi + take]
                fin = [g["rt"] for g in grps if g["stop"]]
                steps.append({"cc": ch, "idx_off": off0 + gi * RT,
                              "n": take * RT, "groups": grps, "fin": fin})
                gi += take
    assert pos == NIDX and gpos == NG

    per_core = []
    for c in range(NC):
        meta = np.zeros((128, NG, 2), dtype=np.float32)
        meta[:, :, 0] = gridx[c].T
        meta[:, :, 1] = gval[c].T
        per_core.append({"gidx": _wrap16(gidx[c]), "meta": meta})

    prog = {"NIDX": NIDX, "NG": NG, "steps": steps}
    return prog, per_core


def build_nc(cfg, prog, ag_mode="collective", ag_split=1):
    import concourse.bacc as bacc
    import concourse.mybir as mybir
    import concourse.tile as tile

    NC, VSP, VG, D, RT, NT = (cfg["NC"], cfg["VSP"], cfg["VG"], cfg["D"],
                              cfg["RT"], cfg["NT"])
    CHUNK, B, FOUT, NQ = cfg["CHUNK"], cfg["B"], cfg["FOUT"], cfg["NQ"]
    NIDX, NG = prog["NIDX"], prog["NG"]
    f32, bf16, i16 = mybir.dt.float32, mybir.dt.bfloat16, mybir.dt.int16
    AG_GROUPS = [list(range(NC))]
    # AllGather split boundaries (by row tile)
    ag_bounds = [NT * (i + 1) // ag_split for i in range(ag_split)]

    nc = bacc.Bacc("TRN2", target_bir_lowering=False, debug=False,
                   num_devices=NC, num_swdge_queues=NQ,
                   dynamic_dma_scratch_size=cfg["SCRATCH"])

    # inputs
    xg0 = nc.dram_tensor("xg0", [VG, D], bf16, kind="ExternalInput")
    x0s = nc.dram_tensor("x0s", [VSP, D], f32, kind="ExternalInput")
    x0t = nc.dram_tensor("x0t", [D, VSP], f32, kind="ExternalInput")
    gidx_d = nc.dram_tensor("gidx", [128, NIDX // 16], i16, kind="ExternalInput")
    meta_d = nc.dram_tensor("meta", [128, NG, 2], f32, kind="ExternalInput")
    iota_d = nc.dram_tensor("iota", [128, RT], bf16, kind="ExternalInput")
    ident_d = nc.dram_tensor("ident", [128, 128], f32, kind="ExternalInput")
    w0_d = nc.dram_tensor("w0", [cfg["FIN"], FOUT], f32, kind="ExternalInput")
    wb_d = nc.dram_tensor("wb", [cfg["FIN"], 3, FOUT], bf16, kind="ExternalInput")
    bias_d = nc.dram_tensor("biasin", [FOUT, 1], f32, kind="ExternalInput")

    # outputs
    outT = nc.dram_tensor("outT", [B, FOUT, VSP], f32, kind="ExternalOutput")

    # internal DRAM
    xb = [nc.dram_tensor(f"xb{k}", [VSP, D], bf16) for k in (1, 2)]
    xf = [nc.dram_tensor(f"xf{k}", [VG, D], bf16, addr_space="Shared")
          for k in (1, 2)]
    xt = [nc.dram_tensor(f"xt{k}", [D, VSP], bf16) for k in (1, 2, 3)]
    xs1 = nc.dram_tensor("xs1", [VSP, D], f32)

    qn = [0]

    with tile.TileContext(nc) as tc:
        with (
            tc.tile_pool(name="static", bufs=1) as sp,
            tc.tile_pool(name="zqp", bufs=6) as zqp,
            tc.tile_pool(name="stile", bufs=8) as stp,
            tc.tile_pool(name="fin", bufs=4) as fp,
            tc.tile_pool(name="psum", bufs=1, space="PSUM") as pp,
            tc.tile_pool(name="pab", bufs=3, space="PSUM") as pabp,
        ):
            gidx_t = sp.tile([128, NIDX // 16], i16)
            meta_t = sp.tile([128, NG, 2], f32)
            iota_t = sp.tile([128, RT], bf16)
            ident_t = sp.tile([128, 128], f32)
            nc.sync.dma_start(out=gidx_t[:], in_=gidx_d[:])
            nc.sync.dma_start(out=meta_t[:], in_=meta_d[:])
            nc.sync.dma_start(out=iota_t[:], in_=iota_d[:])
            nc.sync.dma_start(out=ident_t[:], in_=ident_d[:])

            def finalize(r, t, ps):
                xnew = fp.tile([128, D], f32, tag="xnew", name=f"xnew_{r}_{t}")
                if r == 1:
                    nc.vector.tensor_copy(xnew[:], ps[:])
                else:
                    xprev_src = x0s if r == 2 else xs1
                    xp = fp.tile([128, D], f32, tag="xp", name=f"xp_{r}_{t}")
                    nc.sync.dma_start(
                        out=xp[:], in_=xprev_src[t * RT:(t + 1) * RT, :])
                    nc.vector.scalar_tensor_tensor(
                        xnew[:], ps[:], 2.0, xp[:],
                        op0=mybir.AluOpType.mult,
                        op1=mybir.AluOpType.subtract)
                if r == 1:
                    nc.sync.dma_start(
                        out=xs1[t * RT:(t + 1) * RT, :], in_=xnew[:])
                if r <= 2:
                    xbt = fp.tile([128, D], bf16, tag="xbt",
                                  name=f"xbt_{r}_{t}")
                    nc.any.tensor_copy(xbt[:], xnew[:])
                    nc.sync.dma_start(
                        out=xb[r - 1][t * RT:(t + 1) * RT, :], in_=xbt[:])
                # feature-major copy for the einsum: feature row 2p+j is
                # partition p of plane j
                pab = pabp.tile([128, 256], f32, tag="pab",
                                name=f"pab_{r}_{t}")
                xe = xnew[:].rearrange("v (f two) -> v two f", two=2)
                nc.tensor.transpose(pab[:, 0:128], xe[:, 0, :], ident_t[:])
                nc.tensor.transpose(pab[:, 128:256], xe[:, 1, :], ident_t[:])
                xtp = fp.tile([128, 2, 128], bf16, tag="xtp",
                              name=f"xtp_{r}_{t}")
                nc.any.tensor_copy(
                    xtp[:].rearrange("p two v -> p (two v)"), pab[:])
                nc.sync.dma_start(
                    out=xt[r - 1].rearrange(
                        "(f two) v -> f two v", two=2)[:, :, t * RT:(t + 1) * RT],
                    in_=xtp[:])

            def allgather(r, part):
                lo = 0 if part == 0 else ag_bounds[part - 1] * RT
                hi = ag_bounds[part] * RT
                if ag_mode == "collective":
                    nc.gpsimd.collective_compute(
                        "AllGather", mybir.AluOpType.bypass,
                        replica_groups=AG_GROUPS,
                        ins=[xb[r - 1][lo:hi, :]],
                        outs=[xf[r - 1].rearrange(
                            "(c v) d -> c v d", c=NC)[:, lo:hi, :]])
                else:  # single-core timing stand-in: same DRAM traffic
                    for c in range(NC):
                        nc.sync.dma_start(
                            out=xf[r - 1][c * VSP + lo:c * VSP + hi, :],
                            in_=xb[r - 1][lo:hi, :])

            def spmm_round(r):
                src = xg0 if r == 1 else xf[r - 2]
                live_ps = {}
                pair_bank = {}
                fin_done = 0
                ag_next = 0
                for step in prog["steps"]:
                    ch = step["cc"]
                    n = step["n"]
                    io = step["idx_off"]
                    zq = zqp.tile([128, n // 128, D], bf16, tag="zq",
                                  name=f"zq_{r}_{io}")
                    nc.gpsimd.dma_gather(
                        zq[:], src[ch * CHUNK:(ch + 1) * CHUNK, :],
                        gidx_t[:, io // 16:(io + n) // 16],
                        num_idxs=n, num_idxs_reg=n, elem_size=D,
                        queue_num=qn[0] % NQ)
                    qn[0] += 1
                    for j, grp in enumerate(step["groups"]):
                        g, t = grp["g"], grp["rt"]
                        if grp["start"]:
                            live_ps[t] = pp.tile(
                                [128, D], f32, tag=f"ps{t % cfg['TBLK']}",
                                name=f"ps_{r}_{t}")
                        st = stp.tile([128, RT], bf16, tag="st",
                                      name=f"st_{r}_{g}")
                        nc.any.tensor_scalar(
                            st[:], iota_t[:],
                            meta_t[:, g, 0:1], meta_t[:, g, 1:2],
                            op0=mybir.AluOpType.is_equal,
                            op1=mybir.AluOpType.mult)
                        nc.tensor.matmul(
                            live_ps[t][:], st[:], zq[:, j, :],
                            start=grp["start"], stop=grp["stop"])
                    for t in step["fin"]:
                        finalize(r, t, live_ps.pop(t))
                        fin_done += 1
                        if (r <= 2 and ag_next < len(ag_bounds)
                                and fin_done == ag_bounds[ag_next]):
                            allgather(r, ag_next)
                            ag_next += 1
                assert fin_done == NT and not live_ps

            for r in (1, 2, 3):
                spmm_round(r)

        # einsum: outT[b][o, v] = sum_k W_k^T @ T_k^T[b-rows, v] + bias
        with (
            tc.tile_pool(name="ew", bufs=1) as ewp,
            tc.tile_pool(name="erhs", bufs=3) as erp,
            tc.tile_pool(name="eout", bufs=3) as eop,
            tc.tile_pool(name="epsum", bufs=1, space="PSUM") as epp,
        ):
            w0_t = ewp.tile([cfg["FIN"], FOUT], f32)
            wb_t = ewp.tile([cfg["FIN"], 3, FOUT], bf16)
            bias_t = ewp.tile([FOUT, 1], f32)
            nc.sync.dma_start(out=w0_t[:], in_=w0_d[:])
            nc.sync.dma_start(out=wb_t[:], in_=wb_d[:])
            nc.sync.dma_start(out=bias_t[:], in_=bias_d[:])
            VC = 512
            nvc = (VSP + VC - 1) // VC
            for v in range(nvc):
                v0 = v * VC
                vn = min(VC, VSP - v0)
                for bb in range(B):
                    f0 = bb * cfg["FIN"]
                    r0 = erp.tile([cfg["FIN"], VC], f32, tag="r0",
                                  name=f"r0_{v}_{bb}")
                    nc.sync.dma_start(
                        out=r0[:, :vn], in_=x0t[f0:f0 + cfg["FIN"], v0:v0 + vn])
                    rk = {}
                    for k in (1, 2, 3):
                        rt_ = erp.tile([cfg["FIN"], VC], bf16, tag=f"rk{k}",
                                       name=f"rk_{v}_{bb}_{k}")
                        nc.sync.dma_start(
                            out=rt_[:, :vn],
                            in_=xt[k - 1][f0:f0 + cfg["FIN"], v0:v0 + vn])
                        rk[k] = rt_
                    ops = epp.tile([FOUT, VC], f32, tag=f"eps{bb % 4}",
                                   name=f"eps_{v}_{bb}")
                    nc.tensor.matmul(ops[:, :vn], w0_t[:], r0[:, :vn],
                                     start=True, stop=False)
                    for k in (1, 2, 3):
                        nc.tensor.matmul(ops[:, :vn], wb_t[:, k - 1, :],
                                         rk[k][:, :vn],
                                         start=False, stop=(k == 3))
                    ot = eop.tile([FOUT, VC], f32, tag="ot",
                                  name=f"ot_{v}_{bb}")
                    nc.vector.tensor_scalar(
                        ot[:, :vn], ops[:, :vn], bias_t[:], None,
                        op0=mybir.AluOpType.add)
                    nc.sync.dma_start(out=outT[bb][:, v0:v0 + vn], in_=ot[:, :vn])

    nc.compile()
    return nc


def _host_prep(x, weight, bias, lap_vals, lap_rows, lap_cols, cfg):
    NC, VS, VSP, VG, D = cfg["NC"], cfg["VS"], cfg["VSP"], cfg["VG"], cfg["D"]
    V = cfg["V"]
    x = np.asarray(x, dtype=np.float32)
    x0 = np.ascontiguousarray(x.transpose(1, 0, 2).reshape(V, D))  # [V, B*FIN]

    prog, per_core = preprocess(lap_rows, lap_cols, lap_vals, cfg)

    x0p = np.zeros((VG, D), dtype=np.float32)
    for c in range(NC):
        x0p[c * VSP:c * VSP + VS] = x0[c * VS:(c + 1) * VS]
    xg0 = x0p.astype(BF16)

    iota = np.tile(np.arange(cfg["RT"], dtype=np.float32).astype(BF16), (128, 1))
    ident = np.eye(128, dtype=np.float32)
    weight = np.asarray(weight, dtype=np.float32)
    w0 = weight[0]
    wb = np.zeros((cfg["FIN"], 3, cfg["FOUT"]), dtype=BF16)
    for k in (1, 2, 3):
        wb[:, k - 1] = weight[k].astype(BF16)
    bias_in = np.asarray(bias, dtype=np.float32).reshape(cfg["FOUT"], 1)

    in_maps = []
    for c in range(NC):
        x0sh = x0p[c * VSP:(c + 1) * VSP]
        in_maps.append({
            "xg0": xg0,
            "x0s": np.ascontiguousarray(x0sh),
            "x0t": np.ascontiguousarray(x0sh.T),
            "gidx": per_core[c]["gidx"],
            "meta": per_core[c]["meta"],
            "iota": iota,
            "ident": ident,
            "w0": w0,
            "wb": wb,
            "biasin": bias_in,
        })
    return prog, in_maps


def _assemble(results, cfg):
    NC, VS, VSP, B, FOUT, V = (cfg["NC"], cfg["VS"], cfg["VSP"], cfg["B"],
                               cfg["FOUT"], cfg["V"])
    out = np.empty((B, V, FOUT), dtype=np.float32)
    for c in range(NC):
        oT = np.asarray(results[c]["outT"]).reshape(B, FOUT, VSP)
        out[:, c * VS:(c + 1) * VS, :] = oT.transpose(0, 2, 1)[:, :VS, :]
    return out


def run(x, weight, bias, lap_vals, lap_rows, lap_cols, trace=False):
    """Returns (output, BassKernelResults)."""
    from concourse import bass_utils

    cfg = make_cfg()
    prog, in_maps = _host_prep(x, weight, bias, lap_vals, lap_rows, lap_cols, cfg)
    nc = build_nc(cfg, prog)
    res = bass_utils.run_bass_kernel_spmd(nc, in_maps, list(range(cfg["NC"])),
                                          trace=trace)
    return _assemble(res.results, cfg), res


def kernel(x, weight, bias, lap_vals, lap_rows, lap_cols):
    out, _ = run(x, weight, bias, lap_vals, lap_rows, lap_cols)
    return out


# revision 31
# speedup vs baseline: 22.6590x; 2.1240x over previous
"""ChebConv (K=4) GNN layer on 8 Trainium2 NeuronCores.

Strategy (sharding_hint: row-partition of the Laplacian + replicated weight):
  - Nodes V row-sharded across 8 cores (VS rows each, padded to VSP=NT*128).
  - Each core owns the edges whose destination row lives in its shard.
  - The current poly y lives as a full node-major bf16 replica [VG, 256] in
    DRAM (AllGather of the 8 shards per spmm round). Edge gathers use
    GPSIMD-issued SWDGE dma_gather: each index pulls one node's 512B feature
    row from HBM straight into SBUF at partition i%128, slot i//128 — i.e.
    a [128 edges, 256 feat] matmul-ready block, no transposes.
  - Edges are ordered tile-major: for each block of TBLK row tiles, for each
    of the NCC=4 node chunks (int16 gather index range), the (tile, chunk)
    edge cells. Segment-sum is a one-hot matmul S[e, r]=val*(rloc==r)
    accumulated in a per-tile PSUM bank across the whole stream.
  - A row tile finalizes right after its last matmul: Chebyshev recurrence
    on the vector engine, bf16 shard write (AllGather input), and a pair of
    PE transposes producing the feature-major copy the final einsum needs.
    The AllGather is split in two halves so the first fires mid-round.
  - Final einsum contracts T_k with the replicated weight on the PE.

The instruction stream is identical on all cores (SPMD): per-(tile, chunk)
edge-cell sizes are padded to the max across cores, so only the index /
one-hot data differs per core.
"""

import sys

import numpy as np

sys.path.insert(0, "/opt/trn_rl_repo")

import ml_dtypes  # noqa: E402

BF16 = ml_dtypes.bfloat16


def make_cfg(V=100000, E=1600000, B=4, FIN=64, FOUT=64, NC=8, RT=128,
             NCC=4, TBLK=4, CALLMAX=1024, NQ=4, SCRATCH=65536):
    VS = V // NC
    assert VS * NC == V
    VSP = ((VS + RT - 1) // RT) * RT
    NT = VSP // RT
    VG = VSP * NC
    CHUNK = VG // NCC          # nodes per gather chunk (int16 idx range)
    assert CHUNK * NCC == VG
    assert CHUNK <= 32768      # int16 dma_gather row index limit
    D = B * FIN
    assert CALLMAX <= SCRATCH // 16 // NQ  # per-queue SWDGE ring capacity
    return dict(V=V, E=E, B=B, FIN=FIN, FOUT=FOUT, NC=NC, RT=RT,
                CHUNK=CHUNK, CALLMAX=CALLMAX, VS=VS, VSP=VSP, NT=NT, VG=VG,
                NCC=NCC, TBLK=TBLK, D=D, NQ=NQ, SCRATCH=SCRATCH)


def _wrap16(idx, npart=128):
    """Pack an idx list (len n, multiple of 16) into the SWDGE gather layout:
    idx i at partition i%16, slot i//16, replicated to all 16-partition
    groups."""
    n = idx.shape[0]
    w = idx.reshape(n // 16, 16).T  # [16, n/16]
    return np.tile(w, (npart // 16, 1))


def preprocess(rows, cols, vals, cfg):
    """Build the static SPMD schedule + per-core index/one-hot data.

    Returns (prog, per_core): prog is core-independent structure;
    per_core[c] has 'gidx' [128, NIDX/16] int16 and 'meta' [128, NG, 2] f32.
    """
    NC, VS, VSP, RT, NT = cfg["NC"], cfg["VS"], cfg["VSP"], cfg["RT"], cfg["NT"]
    CHUNK, CALLMAX, NCC, TBLK = (cfg["CHUNK"], cfg["CALLMAX"], cfg["NCC"],
                                 cfg["TBLK"])

    rows = np.asarray(rows, dtype=np.int64)
    cols = np.asarray(cols, dtype=np.int64)
    vals = np.asarray(vals, dtype=np.float32)

    owner = rows // VS
    lr = rows - owner * VS
    rt = lr // RT
    rloc = lr - rt * RT
    gc = (cols // VS) * VSP + (cols % VS)   # padded-global replica row
    cc = gc // CHUNK
    ci = (gc - cc * CHUNK).astype(np.int64)  # row idx within chunk

    # per-core edge cells keyed by (rt, cc)
    cell_of = rt * NCC + cc
    ncells = NT * NCC
    counts = np.zeros((NC, ncells), dtype=np.int64)
    for c in range(NC):
        m = owner == c
        counts[c] = np.bincount(cell_of[m], minlength=ncells)
    mx = counts.max(axis=0)
    mpad = ((mx + RT - 1) // RT) * RT  # padded cell size, common to all cores
    mpad2 = mpad.reshape(NT, NCC)
    # every tile needs at least one group so its PSUM accumulator exists
    for t in range(NT):
        if mpad2[t].sum() == 0:
            mpad2[t, 0] = RT

    per_core_cells = []
    for c in range(NC):
        m = owner == c
        order = np.argsort(cell_of[m], kind="stable")
        e_ci = ci[m][order]
        e_rloc = rloc[m][order]
        e_val = vals[m][order]
        e_cell = cell_of[m][order]
        starts = np.searchsorted(e_cell, np.arange(ncells))
        ends = np.searchsorted(e_cell, np.arange(ncells) + 1)
        per_core_cells.append((e_ci, e_rloc, e_val, starts, ends))

    NIDX = int(mpad2.sum())
    NG = NIDX // RT

    gidx = [np.zeros(NIDX, dtype=np.int16) for _ in range(NC)]
    gridx = [np.zeros((NG, RT), dtype=np.float32) for _ in range(NC)]
    gval = [np.zeros((NG, RT), dtype=np.float32) for _ in range(NC)]

    # per-tile first/last group for start/stop flags
    tile_ngroups = (mpad2 // RT).sum(axis=1)
    steps = []
    pos = 0
    gpos = 0
    for tb in range(0, NT, TBLK):
        tiles = list(range(tb, min(tb + TBLK, NT)))
        remaining = {t: int(tile_ngroups[t]) for t in tiles}
        started = set()
        for ch in range(NCC):
            # groups of cells (t, ch) for t in tiles, in tile order
            seg_groups = []
            for t in tiles:
                n = int(mpad2[t, ch])
                if n == 0:
                    continue
                for c in range(NC):
                    e_ci, e_rloc, e_val, starts_, ends_ = per_core_cells[c]
                    s_, e_ = starts_[t * NCC + ch], ends_[t * NCC + ch]
                    k = e_ - s_
                    gidx[c][pos:pos + k] = e_ci[s_:e_].astype(np.int16)
                    gr = gridx[c][gpos:gpos + n // RT].reshape(-1)
                    gv = gval[c][gpos:gpos + n // RT].reshape(-1)
                    gr[:k] = e_rloc[s_:e_].astype(np.float32)
                    gv[:k] = e_val[s_:e_].astype(np.float32)
                ngr = n // RT
                for j in range(ngr):
                    st = t not in started
                    started.add(t)
                    remaining[t] -= 1
                    seg_groups.append({"g": gpos + j, "rt": t, "start": st,
                                       "stop": remaining[t] == 0})
                pos += n
                gpos += ngr
            # pack groups into calls of <= CALLMAX idxs
            gi = 0
            off0 = pos - len(seg_groups) * RT
            while gi < len(seg_groups):
                take = min(CALLMAX // RT, len(seg_groups) - gi)
                grps = seg_groups[gi:gi + take]
                fin = [g["rt"] for g in grps if g["stop"]]
                steps.append({"cc": ch, "idx_off": off0 + gi * RT,
                              "n": take * RT, "groups": grps, "fin": fin})
                gi += take
    assert pos == NIDX and gpos == NG

    per_core = []
    for c in range(NC):
        meta = np.zeros((128, NG, 2), dtype=np.float32)
        meta[:, :, 0] = gridx[c].T
        meta[:, :, 1] = gval[c].T
        per_core.append({"gidx": _wrap16(gidx[c]), "meta": meta})

    prog = {"NIDX": NIDX, "NG": NG, "steps": steps}
    return prog, per_core


def build_nc(cfg, prog, ag_mode="collective", ag_split=1):
    import concourse.bacc as bacc
    import concourse.mybir as mybir
    import concourse.tile as tile

    NC, VSP, VG, D, RT, NT = (cfg["NC"], cfg["VSP"], cfg["VG"], cfg["D"],
                              cfg["RT"], cfg["NT"])
    CHUNK, B, FOUT, NQ = cfg["CHUNK"], cfg["B"], cfg["FOUT"], cfg["NQ"]
    NIDX, NG = prog["NIDX"], prog["NG"]
    f32, bf16, i16 = mybir.dt.float32, mybir.dt.bfloat16, mybir.dt.int16
    AG_GROUPS = [list(range(NC))]
    # AllGather split boundaries (by row tile)
    ag_bounds = [NT * (i + 1) // ag_split for i in range(ag_split)]

    nc = bacc.Bacc("TRN2", target_bir_lowering=False, debug=False,
                   num_devices=NC, num_swdge_queues=NQ,
                   dynamic_dma_scratch_size=cfg["SCRATCH"])

    # inputs
    xg0 = nc.dram_tensor("xg0", [VG, D], bf16, kind="ExternalInput")
    x0s = nc.dram_tensor("x0s", [VSP, D], f32, kind="ExternalInput")
    x0t = nc.dram_tensor("x0t", [D, VSP], f32, kind="ExternalInput")
    gidx_d = nc.dram_tensor("gidx", [128, NIDX // 16], i16, kind="ExternalInput")
    meta_d = nc.dram_tensor("meta", [128, NG, 2], f32, kind="ExternalInput")
    iota_d = nc.dram_tensor("iota", [128, RT], bf16, kind="ExternalInput")
    ident_d = nc.dram_tensor("ident", [128, 128], f32, kind="ExternalInput")
    w0_d = nc.dram_tensor("w0", [cfg["FIN"], FOUT], f32, kind="ExternalInput")
    wb_d = nc.dram_tensor("wb", [cfg["FIN"], 3, FOUT], bf16, kind="ExternalInput")
    bias_d = nc.dram_tensor("biasin", [FOUT, 1], f32, kind="ExternalInput")

    # outputs
    outT = nc.dram_tensor("outT", [B, FOUT, VSP], f32, kind="ExternalOutput")

    # internal DRAM
    xb = [nc.dram_tensor(f"xb{k}", [VSP, D], bf16) for k in (1, 2)]
    xf = [nc.dram_tensor(f"xf{k}", [VG, D], bf16, addr_space="Shared")
          for k in (1, 2)]
    xt = [nc.dram_tensor(f"xt{k}", [D, VSP], bf16) for k in (1, 2, 3)]

    qn = [0]

    with tile.TileContext(nc) as tc:
        with (
            tc.tile_pool(name="static", bufs=1) as sp,
            tc.tile_pool(name="zqp", bufs=6) as zqp,
            tc.tile_pool(name="stile", bufs=8) as stp,
            tc.tile_pool(name="fin", bufs=4) as fp,
            tc.tile_pool(name="psum", bufs=1, space="PSUM") as pp,
            tc.tile_pool(name="pab", bufs=3, space="PSUM") as pabp,
        ):
            gidx_t = sp.tile([128, NIDX // 16], i16)
            meta_t = sp.tile([128, NG, 2], f32)
            iota_t = sp.tile([128, RT], bf16)
            ident_t = sp.tile([128, 128], f32)
            ident_bf = sp.tile([128, 128], bf16)
            xprev_sb = sp.tile([128, NT, D], bf16)  # x1 kept on-chip for r=3
            nc.sync.dma_start(out=gidx_t[:], in_=gidx_d[:])
            nc.sync.dma_start(out=meta_t[:], in_=meta_d[:])
            nc.sync.dma_start(out=iota_t[:], in_=iota_d[:])
            nc.sync.dma_start(out=ident_t[:], in_=ident_d[:])
            nc.vector.tensor_copy(ident_bf[:], ident_t[:])

            def finalize(r, t, ps):
                xnew = fp.tile([128, D], f32, tag="xnew",
                               name=f"xnew_{r}_{t}")[:]
                if r == 1:
                    nc.vector.tensor_copy(xnew, ps[:])
                else:
                    if r == 2:
                        xp = fp.tile([128, D], f32, tag="xp",
                                     name=f"xp_{r}_{t}")[:]
                        nc.scalar.dma_start(
                            out=xp, in_=x0s[t * RT:(t + 1) * RT, :])
                    else:
                        xp = xprev_sb[:, t, :]
                    nc.vector.scalar_tensor_tensor(
                        xnew, ps[:], 2.0, xp,
                        op0=mybir.AluOpType.mult,
                        op1=mybir.AluOpType.subtract)
                if r == 1:
                    nc.any.tensor_copy(xprev_sb[:, t, :], xnew)
                    nc.scalar.dma_start(
                        out=xb[0][t * RT:(t + 1) * RT, :],
                        in_=xprev_sb[:, t, :])
                elif r == 2:
                    xbt = fp.tile([128, D], bf16, tag="xbt",
                                  name=f"xbt_{r}_{t}")
                    nc.any.tensor_copy(xbt[:], xnew)
                    nc.scalar.dma_start(
                        out=xb[1][t * RT:(t + 1) * RT, :], in_=xbt[:])
                # feature-major copy for the einsum: feature row 2p+j is
                # partition p of plane j
                pab = pabp.tile([128, 256], f32, tag="pab",
                                name=f"pab_{r}_{t}")
                xe = xnew.rearrange("v (f two) -> v two f", two=2)
                nc.tensor.transpose(pab[:, 0:128], xe[:, 0, :], ident_t[:])
                nc.tensor.transpose(pab[:, 128:256], xe[:, 1, :], ident_t[:])
                xtp = fp.tile([128, 2, 128], bf16, tag="xtp",
                              name=f"xtp_{r}_{t}")
                nc.any.tensor_copy(
                    xtp[:].rearrange("p two v -> p (two v)"), pab[:])
                nc.scalar.dma_start(
                    out=xt[r - 1].rearrange(
                        "(f two) v -> f two v", two=2)[:, :, t * RT:(t + 1) * RT],
                    in_=xtp[:])

            def allgather(r, part):
                lo = 0 if part == 0 else ag_bounds[part - 1] * RT
                hi = ag_bounds[part] * RT
                if ag_mode == "collective":
                    nc.gpsimd.collective_compute(
                        "AllGather", mybir.AluOpType.bypass,
                        replica_groups=AG_GROUPS,
                        ins=[xb[r - 1][lo:hi, :]],
                        outs=[xf[r - 1].rearrange(
                            "(c v) d -> c v d", c=NC)[:, lo:hi, :]])
                else:  # single-core timing stand-in: same DRAM traffic
                    for c in range(NC):
                        nc.sync.dma_start(
                            out=xf[r - 1][c * VSP + lo:c * VSP + hi, :],
                            in_=xb[r - 1][lo:hi, :])

            def spmm_round(r):
                src = xg0 if r == 1 else xf[r - 2]
                live_ps = {}
                pair_bank = {}
                fin_done = 0
                ag_next = 0
                for step in prog["steps"]:
                    ch = step["cc"]
                    n = step["n"]
                    io = step["idx_off"]
                    zq = zqp.tile([128, n // 128, D], bf16, tag="zq",
                                  name=f"zq_{r}_{io}")
                    nc.gpsimd.dma_gather(
                        zq[:], src[ch * CHUNK:(ch + 1) * CHUNK, :],
                        gidx_t[:, io // 16:(io + n) // 16],
                        num_idxs=n, num_idxs_reg=n, elem_size=D,
                        queue_num=qn[0] % NQ)
                    qn[0] += 1
                    for j, grp in enumerate(step["groups"]):
                        g, t = grp["g"], grp["rt"]
                        if grp["start"]:
                            live_ps[t] = pp.tile(
                                [128, D], f32, tag=f"ps{t % cfg['TBLK']}",
                                name=f"ps_{r}_{t}")
                        st = stp.tile([128, RT], bf16, tag="st",
                                      name=f"st_{r}_{g}")
                        nc.any.tensor_scalar(
                            st[:], iota_t[:],
                            meta_t[:, g, 0:1], meta_t[:, g, 1:2],
                            op0=mybir.AluOpType.is_equal,
                            op1=mybir.AluOpType.mult)
                        nc.tensor.matmul(
                            live_ps[t][:], st[:], zq[:, j, :],
                            start=grp["start"], stop=grp["stop"])
                    for t in step["fin"]:
                        finalize(r, t, live_ps.pop(t))
                        fin_done += 1
                        if (r <= 2 and ag_next < len(ag_bounds)
                                and fin_done == ag_bounds[ag_next]):
                            allgather(r, ag_next)
                            ag_next += 1
                assert fin_done == NT and not live_ps

            for r in (1, 2, 3):
                spmm_round(r)

        # einsum: outT[b][o, v] = sum_k W_k^T @ T_k^T[b-rows, v] + bias
        with (
            tc.tile_pool(name="ew", bufs=1) as ewp,
            tc.tile_pool(name="erhs", bufs=3) as erp,
            tc.tile_pool(name="eout", bufs=3) as eop,
            tc.tile_pool(name="epsum", bufs=1, space="PSUM") as epp,
        ):
            w0_t = ewp.tile([cfg["FIN"], FOUT], f32)
            wb_t = ewp.tile([cfg["FIN"], 3, FOUT], bf16)
            bias_t = ewp.tile([FOUT, 1], f32)
            nc.sync.dma_start(out=w0_t[:], in_=w0_d[:])
            nc.sync.dma_start(out=wb_t[:], in_=wb_d[:])
            nc.sync.dma_start(out=bias_t[:], in_=bias_d[:])
            VC = 512
            nvc = (VSP + VC - 1) // VC
            for v in range(nvc):
                v0 = v * VC
                vn = min(VC, VSP - v0)
                for bb in range(B):
                    f0 = bb * cfg["FIN"]
                    r0 = erp.tile([cfg["FIN"], VC], f32, tag="r0",
                                  name=f"r0_{v}_{bb}")
                    nc.sync.dma_start(
                        out=r0[:, :vn], in_=x0t[f0:f0 + cfg["FIN"], v0:v0 + vn])
                    rk = {}
                    for k in (1, 2, 3):
                        rt_ = erp.tile([cfg["FIN"], VC], bf16, tag=f"rk{k}",
                                       name=f"rk_{v}_{bb}_{k}")
                        nc.scalar.dma_start(
                            out=rt_[:, :vn],
                            in_=xt[k - 1][f0:f0 + cfg["FIN"], v0:v0 + vn])
                        rk[k] = rt_
                    ops = epp.tile([FOUT, VC], f32, tag=f"eps{bb % 4}",
                                   name=f"eps_{v}_{bb}")
                    nc.tensor.matmul(ops[:, :vn], w0_t[:], r0[:, :vn],
                                     start=True, stop=False)
                    for k in (1, 2, 3):
                        nc.tensor.matmul(ops[:, :vn], wb_t[:, k - 1, :],
                                         rk[k][:, :vn],
                                         start=False, stop=(k == 3))
                    ot = eop.tile([FOUT, VC], f32, tag="ot",
                                  name=f"ot_{v}_{bb}")
                    nc.vector.tensor_scalar(
                        ot[:, :vn], ops[:, :vn], bias_t[:], None,
                        op0=mybir.AluOpType.add)
                    nc.sync.dma_start(out=outT[bb][:, v0:v0 + vn], in_=ot[:, :vn])

    nc.compile()
    return nc


def _host_prep(x, weight, bias, lap_vals, lap_rows, lap_cols, cfg):
    NC, VS, VSP, VG, D = cfg["NC"], cfg["VS"], cfg["VSP"], cfg["VG"], cfg["D"]
    V = cfg["V"]
    x = np.asarray(x, dtype=np.float32)
    x0 = np.ascontiguousarray(x.transpose(1, 0, 2).reshape(V, D))  # [V, B*FIN]

    prog, per_core = preprocess(lap_rows, lap_cols, lap_vals, cfg)

    x0p = np.zeros((VG, D), dtype=np.float32)
    for c in range(NC):
        x0p[c * VSP:c * VSP + VS] = x0[c * VS:(c + 1) * VS]
    xg0 = x0p.astype(BF16)

    iota = np.tile(np.arange(cfg["RT"], dtype=np.float32).astype(BF16), (128, 1))
    ident = np.eye(128, dtype=np.float32)
    weight = np.asarray(weight, dtype=np.float32)
    w0 = weight[0]
    wb = np.zeros((cfg["FIN"], 3, cfg["FOUT"]), dtype=BF16)
    for k in (1, 2, 3):
        wb[:, k - 1] = weight[k].astype(BF16)
    bias_in = np.asarray(bias, dtype=np.float32).reshape(cfg["FOUT"], 1)

    in_maps = []
    for c in range(NC):
        x0sh = x0p[c * VSP:(c + 1) * VSP]
        in_maps.append({
            "xg0": xg0,
            "x0s": np.ascontiguousarray(x0sh),
            "x0t": np.ascontiguousarray(x0sh.T),
            "gidx": per_core[c]["gidx"],
            "meta": per_core[c]["meta"],
            "iota": iota,
            "ident": ident,
            "w0": w0,
            "wb": wb,
            "biasin": bias_in,
        })
    return prog, in_maps


def _assemble(results, cfg):
    NC, VS, VSP, B, FOUT, V = (cfg["NC"], cfg["VS"], cfg["VSP"], cfg["B"],
                               cfg["FOUT"], cfg["V"])
    out = np.empty((B, V, FOUT), dtype=np.float32)
    for c in range(NC):
        oT = np.asarray(results[c]["outT"]).reshape(B, FOUT, VSP)
        out[:, c * VS:(c + 1) * VS, :] = oT.transpose(0, 2, 1)[:, :VS, :]
    return out


def run(x, weight, bias, lap_vals, lap_rows, lap_cols, trace=False):
    """Returns (output, BassKernelResults)."""
    from concourse import bass_utils

    cfg = make_cfg()
    prog, in_maps = _host_prep(x, weight, bias, lap_vals, lap_rows, lap_cols, cfg)
    nc = build_nc(cfg, prog)
    res = bass_utils.run_bass_kernel_spmd(nc, in_maps, list(range(cfg["NC"])),
                                          trace=trace)
    return _assemble(res.results, cfg), res


def kernel(x, weight, bias, lap_vals, lap_rows, lap_cols):
    out, _ = run(x, weight, bias, lap_vals, lap_rows, lap_cols)
    return out
